# revision 75
# baseline (speedup 1.0000x reference)
"""Trainium2 Bass kernel for nn_EnergyLoss: batched 16x16 complex Hermitian
ground-state projector via shifted matrix-squaring power iteration.

Math summary (all derived from the reference):
  H[n] = 0.5*G - 0.5*sum_d X[n,d]*S_d + (0.5*q_n + EPS)*I,
     G = sum_d A_d A_d^H,  S_d = A_d + A_d^H,  q_n = sum_d X[n,d]^2
  B0 = I - H/(C_SHIFT*||H||_F)   (C_SHIFT tuned so f > (lmax+l1)/2 for every
     sample => |mu(l0)| dominates and the iteration converges to the ground
     state; c<1 tightens the spectral gap and cuts squaring steps)
  B <- B^2, renormalized by tr(B^2)=||B||_F^2 on steps E={1,3,5,7}; the
     normalizer is computed from the state one step EARLIER (on the idle
     N-steps) so the E-step cast never waits on a reduce.
  loss terms from the projector via rowsums (see WPOS/WEA2).
Complex 16x16 matrices are embedded as real 32x32 M(B)=[[Br,-Bi],[Bi,Br]];
per-sample squaring runs as 32x32 PE-array tile matmuls (4 samples per 128
partitions, diagonal tile_positions).  State is fp16, PSUM fp32.

Per-sample 1/(C_SHIFT*fro(H)) is exact on host via the quadratic form
fro^2 = y^T (WH WH^T) y, y=[x;1;q], folded into the XTH columns, so the
device H-matmul directly yields H/f and phase 1 needs no norm pipeline.
Engine split: PE matmuls; DVE psum casts/reduces/shuffles; Act sign-scaled
copies (per-partition scale); Pool state squares + SBUF-only finish ops
(gpsimd cannot touch PSUM or use scalar-broadcast ops on trn2).
"""

import numpy as np

N, D, DIM = 4096, 32, 16
NCORES = 8
NS = N // NCORES          # 512 samples per core
NQ = NS // 4              # 128 quads (4 samples stacked per 128 partitions)
EPS = 1e-5
LAM = 0.1
C_SHIFT = 0.28
KSTEPS = 8
NSLAB = 4                 # quad slabs for pipelining
QS = NQ // NSLAB          # 32 quads per slab

_prog_cache = {}

# packed constant-input byte offsets (per partition), ordered by DMA urgency:
# DMA1 = signs+XTH+WH (H matmuls + first casts), DMA2 = DIAGP (phase-1 STT),
# DMA3 = the rest (first needed by the k=0 normalizer prefetch)
OFF_SIGNP = 0         # f32 [128,1]    4B
OFF_SIGNPM = 4        # f32 [128,1]    4B
OFF_XTH = 16          # f16 [34,512]   1024B  (columns pre-scaled by 1/f_n)
OFF_WH = 1040         # f16 [34,512]   1024B
OFF_DIAGP = 2064      # f16 [128,2048] 4096B
OFF_MASKBS = 6160     # f16 [128,128]  256B   signed block mask (+-1 rows)
OFF_SIGNW = 6416      # f16 [128,512]  1024B  per-partition sign, 512 wide
OFF_XBLK = 7440       # f32 [128,128]  512B
OFF_WPOS = 7952       # f16 [128,128]  256B
OFF_WEA2 = 8208       # f16 [128,128]  256B   (pre-scaled by LAMBDA_REG)
OFF_WTRC = 8464       # f16 [128,16*128] 4096B  per-j trace-pick block masks
OFF_MASKBSF = 12560   # f32 [128,128]  512B   signed mask, f32 for DVE path
OFF_WPOSF = 13072     # f32 [128,128]  512B
OFF_WEA2F = 13584     # f32 [128,128]  512B
CIN_BYTES = 14096
DMA1_HI = OFF_DIAGP
DMA2_HI = OFF_MASKBS
HSWAP = list(range(16, 32)) + list(range(0, 16))


def _build_host_tensors(A_real, A_imag, X):
    """All small A-derived tensors + per-core X-derived layouts (numpy fp32)."""
    A = (A_real + 1j * A_imag).astype(np.complex64)
    Sc = A + np.conj(np.transpose(A, (0, 2, 1)))        # [D,16,16] Hermitian
    Sr, Si = Sc.real.astype(np.float64), Sc.imag.astype(np.float64)
    G = np.einsum('dij,dkj->ik', A, A.conj())
    Gr, Gi = G.real.astype(np.float64), G.imag.astype(np.float64)
    cA = A.sum(axis=1)                                   # [D,16] colsum over i
    cA2 = (A @ A).sum(axis=1)

    # H-build weights: WH[k, 32j+m], contraction k: 0..31 = d, 32 = const, 33 = q
    WH = np.zeros((34, 512), np.float64)
    for j in range(DIM):
        c = 32 * j
        WH[:D, c:c+16] = -0.5 * Sr[:, :, j]              # m<16 -> Hr[m,j]
        WH[:D, c+16:c+32] = -0.5 * Si[:, :, j]           # m>=16 -> Hi[m-16,j]
        WH[32, c:c+16] = 0.5 * Gr[:, j]
        WH[32, c+j] += EPS
        WH[32, c+16:c+32] = 0.5 * Gi[:, j]
        WH[33, c+j] = 0.5
    MQF = WH @ WH.T                                      # fro^2 quadratic form
    # diag delta pattern on the state layout (top halves only)
    DIAGP = np.zeros((128, 16 * NQ), np.float32)
    for s in range(4):
        for i in range(DIM):
            DIAGP[32*s + i, i::16] = 1.0
    # block masks for cross-partition per-sample sums; signed variant bakes
    # the state's [+ / -] partition sign into the broadcast normalizer.
    # WTRC[p, 128j+p'] picks the j-th diagonal row of each block (trace).
    MASKBS = np.zeros((128, 128), np.float32)
    WTRC = np.zeros((128, 16 * 128), np.float32)
    SIGNP = np.ones((128, 1), np.float32)
    for s in range(4):
        SIGNP[32*s+16:32*s+32, 0] = -1.0
    for b in range(4):
        MASKBS[32*b:32*b+32, 32*b:32*b+32] = SIGNP[32*b:32*b+32, 0][None, :]
        for j in range(DIM):
            WTRC[32*b+j, 128*j+32*b:128*j+32*b+32] = 1.0
    # finish functionals: rhs is RS from s2 = [Pr; -Pi] rowsums
    #   pos_raw[32s+d, q] = sum_i cAr[d,i]*rr[i] - cAi[d,i]*ri[i]
    #   RS bottom rows hold -ri  =>  bottom weight = +cAi
    WPOS = np.zeros((128, 128), np.float32)
    WEA2 = np.zeros((128, 128), np.float32)
    for s in range(4):
        b = 32 * s
        WPOS[b:b+16, b:b+32] = cA.real.T                 # [i, d]
        WPOS[b+16:b+32, b:b+32] = cA.imag.T
        WEA2[b:b+16, b:b+32] = LAM * cA2.real.T
        WEA2[b+16:b+32, b:b+32] = LAM * cA2.imag.T

    def put(buf, rows, off, arr):
        b = np.ascontiguousarray(arr).view(np.uint8).reshape(arr.shape[0], -1)
        buf[:rows, off:off+b.shape[1]] = b

    per_core = []
    for c in range(NCORES):
        Xc = np.asarray(X[c*NS:(c+1)*NS], np.float64)    # [512, 32]
        q = (Xc ** 2).sum(1)
        Y = np.concatenate([Xc, np.ones((NS, 1)), q[:, None]], axis=1)  # [512,34]
        fro = np.sqrt(np.einsum('nk,kl,nl->n', Y, MQF, Y))
        invf = 1.0 / (C_SHIFT * fro)                     # [512]
        YS = (Y * invf[:, None]).astype(np.float32)      # scaled y columns
        XTH = np.zeros((34, 512), np.float32)
        XBLK = np.zeros((128, 128), np.float32)
        for s in range(4):
            idx = np.arange(NQ) * 4 + s                  # n_core(q,s)
            XTH[:, 128*s:128*s+128] = YS[idx].T
            XBLK[32*s:32*s+32, :] = Xc[idx].T
        buf = np.zeros((128, CIN_BYTES), np.uint8)
        put(buf, 128, OFF_XBLK, XBLK)
        put(buf, 128, OFF_MASKBS, MASKBS.astype(np.float16))
        put(buf, 128, OFF_SIGNW, np.repeat(SIGNP, 512, 1).astype(np.float16))
        put(buf, 128, OFF_SIGNP, SIGNP)
        put(buf, 128, OFF_SIGNPM, -SIGNP)
        put(buf, 34, OFF_XTH, XTH.astype(np.float16))
        put(buf, 34, OFF_WH, WH.astype(np.float16))
        put(buf, 128, OFF_WPOS, WPOS.astype(np.float16))
        put(buf, 128, OFF_WEA2, WEA2.astype(np.float16))
        put(buf, 128, OFF_WTRC, WTRC.astype(np.float16))
        put(buf, 128, OFF_MASKBSF, MASKBS)
        put(buf, 128, OFF_WPOSF, WPOS)
        put(buf, 128, OFF_WEA2F, WEA2)
        put(buf, 128, OFF_DIAGP, DIAGP.astype(np.float16))
        per_core.append({"cin": buf})
    return per_core


def build_program(ksteps=KSTEPS, warmup=20, PREF_DVE_SLABS=1, WB_PRIO=0):
    import concourse.bass as bass
    import concourse.bacc as bacc
    import concourse.mybir as mybir
    import concourse.tile as tile
    from contextlib import ExitStack

    f16, f32 = mybir.dt.float16, mybir.dt.float32
    u8, u32 = mybir.dt.uint8, mybir.dt.uint32
    Alu = mybir.AluOpType
    Act = mybir.ActivationFunctionType

    nc = bacc.Bacc()
    d_cin = nc.dram_tensor("cin", [128, CIN_BYTES], u8, kind="ExternalInput")
    d_out = nc.dram_tensor("out", [128, 2], f32, kind="ExternalOutput")

    SW = 16 * QS              # state cols per slab (512)

    with tile.TileContext(nc) as tc, ExitStack() as ctx:
        cpool = ctx.enter_context(tc.tile_pool(name="consts", bufs=1))
        spool = ctx.enter_context(tc.tile_pool(name="state", bufs=3))
        wpool = ctx.enter_context(tc.tile_pool(name="work", bufs=2))
        ppool_pm = ctx.enter_context(tc.tile_pool(name="psum_pm", bufs=6, space="PSUM"))
        ppool_sm = ctx.enter_context(tc.tile_pool(name="psum_sm", bufs=2, space="PSUM"))

        # ---------------- input DMAs (critical slice first) -----------------
        cst = cpool.tile([128, CIN_BYTES], u8, tag="cin")
        nc.sync.dma_start(cst[:, 0:DMA1_HI], d_cin[:, 0:DMA1_HI])
        nc.sync.dma_start(cst[:, DMA1_HI:DMA1_HI+1024],
                          d_cin[:, DMA1_HI:DMA1_HI+1024])
        nc.sync.dma_start(cst[:, DMA1_HI+1024:DMA2_HI],
                          d_cin[:, DMA1_HI+1024:DMA2_HI])
        nc.sync.dma_start(cst[:, DMA2_HI:], d_cin[:, DMA2_HI:])
        xblk = cst[:, OFF_XBLK:OFF_XBLK+512].bitcast(f32)
        maskbs = cst[:, OFF_MASKBS:OFF_MASKBS+256].bitcast(f16)
        signw = cst[:, OFF_SIGNW:OFF_SIGNW+1024].bitcast(f16)
        signp = cst[:, OFF_SIGNP:OFF_SIGNP+4].bitcast(f32)
        signpm = cst[:, OFF_SIGNPM:OFF_SIGNPM+4].bitcast(f32)
        xth = cst[:, OFF_XTH:OFF_XTH+1024].bitcast(f16)[0:34, :]
        wh = cst[:, OFF_WH:OFF_WH+1024].bitcast(f16)[0:34, :]
        wpos = cst[:, OFF_WPOS:OFF_WPOS+256].bitcast(f16)
        wea2 = cst[:, OFF_WEA2:OFF_WEA2+256].bitcast(f16)
        wtrc = cst[:, OFF_WTRC:OFF_WTRC+4096].bitcast(f16)
        maskbsf = cst[:, OFF_MASKBSF:OFF_MASKBSF+512].bitcast(f32)
        wposf = cst[:, OFF_WPOSF:OFF_WPOSF+512].bitcast(f32)
        wea2f = cst[:, OFF_WEA2F:OFF_WEA2F+512].bitcast(f32)
        diagp = cst[:, OFF_DIAGP:OFF_DIAGP+4096].bitcast(f16)

        # ---------------- PE p-state warmup during the DMA ------------------
        if warmup:
            wz = wpool.tile([128, 128], f16, tag="warm")
            nc.gpsimd.memset(wz[:, :], 0)
            wz2 = wpool.tile([128, 128], f16, tag="warm2")
            nc.scalar.activation(wz2[:, :], wz[:, :], Act.Copy)
            pwarm = ppool_pm.tile([128, 512], f32, tag="pm")
            for i in range(warmup):
                nc.tensor.matmul(pwarm[:, 128*(i % 4):128*(i % 4)+128],
                                 wz[:, :], wz[:, :], start=True, stop=True)

        # ---------------- phase 1: H/f build, B0 = I - H/f ------------------
        # Quad-sliced by slab so slab 0's state (and its wb + step-0 matmuls)
        # flows while slabs 1-3 are still streaming H.
        s2 = spool.tile([128, 2048], f16, tag="s2")
        p1t = wpool.tile([128, 2048], f16, tag="p1t")
        for sl in range(NSLAB):
            ph = ppool_pm.tile([128, SW], f32, tag="pm")
            for j in range(DIM):
                for s in range(4):
                    nc.tensor.matmul(
                        ph[32*s:32*s+32, QS*j:QS*(j+1)],
                        wh[:, 32*j:32*j+32],
                        xth[:, 128*s+QS*sl:128*s+QS*(sl+1)],
                        start=True, stop=True,
                        tile_position=(0, 32*s),
                    )
            # s2 slab = diagp + pm*signpm: Act does the psum read + sign,
            # DVE adds the diagonal (pipelines across slabs)
            nc.scalar.activation(
                p1t[:, SW*sl:SW*(sl+1)].rearrange("p (q j) -> p q j", j=DIM),
                ph[:, :].rearrange("p (j q) -> p q j", j=DIM),
                Act.Copy, scale=signpm[:, :])
            nc.vector.tensor_tensor(s2[:, SW*sl:SW*(sl+1)],
                                    p1t[:, SW*sl:SW*(sl+1)],
                                    diagp[:, SW*sl:SW*(sl+1)], op=Alu.add)

        def build_wb_left(wb_t, s2_t, sl):
            """wb[:, 32q+0:16] = s2*signp (-> [Br;Bi]).  Alternate slabs go to
            Act (per-partition scale) and Pool (real-tensor multiply by the
            materialized sign plane -- Pool can't use scalar-broadcast ops)."""
            wbl = wb_t[:, :].rearrange("p (q j) -> p q j", j=32)
            src = s2_t[:, SW*sl:SW*(sl+1)].rearrange("p (q j) -> p q j", j=DIM)
            dst = wbl[:, sl*QS:(sl+1)*QS, 0:16]
            if sl % 2 == 0:
                nc.scalar.activation(dst, src, Act.Copy, scale=signp[:, :])
            else:
                nc.gpsimd.tensor_tensor(
                    dst, src,
                    signw[:, 0:SW].rearrange("p (q j) -> p q j", j=DIM),
                    op=Alu.mult)

        def build_wb_right(wb_t, s2_t, sl):
            """wb[:, 32q+16:32] = partition-half-swapped s2 (-> [-Bi;Br])."""
            wbw = wb_t[:, :].bitcast(u32).rearrange("p (q w) -> p q w", w=16)
            s2w = s2_t[:, :].bitcast(u32)
            nc.vector.stream_shuffle(
                wbw[:, sl*QS:(sl+1)*QS, 8:16],
                s2w[:, 8*sl*QS:8*(sl+1)*QS].rearrange("p (q w) -> p q w", w=8),
                mask=HSWAP)

        scl = [None, None]    # per slab-pair: [128, 64] tiles of +-1/s

        def norm_prefetch(s2_t, sl, sq_t, trp_t):
            """Normalizer for the NEXT step's cast: fro^2 of the state slab.
            Square on Pool (SBUF only); the (partition-block x j) double sum
            runs as 16 PSUM-accumulating matmuls over j-strided rhs slices
            with the sign-baked mask weights -- zero DVE cost; one batched
            recip per slab-pair then yields +-1/s."""
            c0, c1 = SW*sl, SW*(sl+1)
            nc.vector.tensor_tensor(sq_t[:, c0:c1], s2_t[:, c0:c1],
                                    s2_t[:, c0:c1], op=Alu.mult)
            slot = sl % 2
            if sl >= PREF_DVE_SLABS:
                sqj = sq_t[:, c0:c1].rearrange("p (q j) -> p j q", j=DIM)
                for j in range(DIM):
                    nc.tensor.matmul(trp_t[:, QS*slot:QS*(slot+1)],
                                     maskbs[:, :], sqj[:, j, :],
                                     start=(j == 0), stop=(j == DIM - 1))
            else:
                pr = wpool.tile([128, QS], f32, tag=f"pr{sl}")
                nc.vector.tensor_reduce(
                    pr[:, :],
                    sq_t[:, c0:c1].rearrange("p (q j) -> p q j", j=DIM),
                    axis=mybir.AxisListType.X, op=Alu.add)
                nc.tensor.matmul(trp_t[:, QS*slot:QS*(slot+1)], maskbsf[:, :],
                                 pr[:, :], start=True, stop=True)
            if slot == 1:
                # recip per slab-pair: ready mid-step, so next step's E-casts
                # for these slabs never wait on the tail slab's prefetch
                hp = sl // 2
                scl_t = wpool.tile([128, 2*QS], f32, tag=f"scl{hp}")
                nc.vector.reciprocal(scl_t[:, :], trp_t[:, :])
                scl[hp] = scl_t

        wb = spool.tile([128, 4096], f16, tag="wb")
        for sl in range(NSLAB):
            if sl % 2 == 0:
                build_wb_left(wb, s2, sl)
            else:
                wbl = wb[:, :].rearrange("p (q j) -> p q j", j=32)
                nc.vector.tensor_scalar_mul(
                    wbl[:, sl*QS:(sl+1)*QS, 0:16],
                    s2[:, SW*sl:SW*(sl+1)].rearrange("p (q j) -> p q j", j=DIM),
                    signp[:, :])
            build_wb_right(wb, s2, sl)

        # ---------------- phase 3: squaring iteration -----------------------
        # E-steps (normalized cast) on odd k; normalizer prefetched on even k.
        for k in range(ksteps):
            last = (k == ksteps - 1)
            # E-steps are STAGGERED by slab-pair (slabs 0-1 normalize on odd
            # steps, slabs 2-3 on even) so every step carries a uniform
            # DVE-cast/Act-cast/prefetch mix; the last step is unnormalized
            # for all slabs (the trace division absorbs the scale)
            s2n = spool.tile([128, 2048], f16, tag="s2")
            wbn = None if last else spool.tile([128, 4096], f16, tag="wb")
            sqn = trpn = None
            if k < ksteps - 2:
                sqn = wpool.tile([128, 2048], f16, tag="sq", name="sqn")
                trpn = ppool_sm.tile([128, 2*QS], f32, tag="sm", name="trpn")
            for sl in range(NSLAB):
                exact = (not last) and k >= 1 and \
                    (k % 2 == (1 if sl < 2 else 0))
                pref = (not last) and k + 1 < ksteps - 1 and \
                    ((k + 1) % 2 == (1 if sl < 2 else 0))
                q0 = sl * QS
                pm = ppool_pm.tile([128, SW], f32, tag="pm")
                for qq in range(QS):
                    q = q0 + qq
                    for s in range(4):
                        nc.tensor.matmul(
                            pm[32*s:32*s+32, 16*qq:16*qq+16],
                            wb[32*s:32*s+32, 32*q:32*q+32],
                            wb[32*s:32*s+32, 32*q:32*q+16],
                            start=True, stop=True,
                            tile_position=(32*s, 32*s))
                dst = s2n[:, SW*sl:SW*(sl+1)].rearrange("p (q j) -> p q j", j=DIM)
                src = pm[:, :].rearrange("p (q j) -> p q j", j=DIM)
                if exact:
                    # cast: s2' = pm * (+-1/s), per-quad scale -> DVE TT
                    nc.vector.tensor_tensor(
                        dst, src,
                        scl[sl // 2][:, QS*(sl % 2):QS*(sl % 2 + 1)]
                            .unsqueeze(-1).broadcast_to([128, QS, DIM]),
                        op=Alu.mult)
                elif last and sl % 2 == 1:
                    # last step: DVE is idle, split the casts with Act
                    nc.vector.tensor_scalar_mul(dst, src, signp[:, :])
                else:
                    # cast: s2' = pm * sign (no normalization this step)
                    nc.scalar.activation(dst, src, Act.Copy, scale=signp[:, :])
                if not last:
                    with tc.high_priority(offset=WB_PRIO):
                        build_wb_left(wbn, s2n, sl)
                        build_wb_right(wbn, s2n, sl)
                    if pref:
                        norm_prefetch(s2n, sl, sqn, trpn)
            s2 = s2n
            if not last:
                wb = wbn

        # ---------------- phase 4: finish (per-slab for pipelining) ---------
        # pos/ea2/trace fold their j-rowsums into PSUM-accumulating matmuls
        # over j-strided state slices: no DVE reduces at all in the tail
        trf = ppool_sm.tile([128, 128], f32, tag="sm")
        pos = ppool_sm.tile([128, 128], f32, tag="sm")
        ea2 = ppool_sm.tile([128, 128], f32, tag="sm")
        def s2j(sl):
            return s2[:, SW*sl:SW*(sl+1)].rearrange("p (q j) -> p j q", j=DIM)
        for w_t, out_t in ((wtrc, trf), (wpos, pos), (wea2, ea2)):
            for sl in range(NSLAB):
                qs = slice(QS*sl, QS*(sl+1))
                for j in range(DIM):
                    wsl = w_t[:, 128*j:128*(j+1)] if w_t is wtrc else w_t[:, :]
                    nc.tensor.matmul(out_t[:, qs], wsl, s2j(sl)[:, j, :],
                                     start=(j == 0), stop=(j == DIM - 1))
        # the rest is column-sliceable; per-slab chains run concurrently on
        # DVE and Pool (PSUM readers stay on DVE; Pool's chain avoids scalar
        # ops -- TensorScalarPtr is illegal there -- via LAMW); the output
        # DMA is split so the first half's latency hides under the second
        invt = wpool.tile([128, 128], f32, tag="invt")
        posn = wpool.tile([128, 128], f32, tag="posn")
        ea2s = wpool.tile([128, 128], f32, tag="ea2s")
        terr = wpool.tile([128, 128], f32, tag="terr")
        t2 = wpool.tile([128, 128], f32, tag="t2")
        np2 = wpool.tile([128, 128], f32, tag="np2")
        r1 = wpool.tile([128, 128], f32, tag="r1")
        r = wpool.tile([128, 128], f32, tag="r")
        outv = wpool.tile([128, 2], f32, tag="outv")
        lamw = wpool.tile([128, 64], f32, tag="lamw")
        nc.gpsimd.memset(lamw[:, :], -LAM)
        for hf in range(2):
            cs = slice(64*hf, 64*hf+64)
            te = nc.vector if hf == 0 else nc.gpsimd
            nc.vector.reciprocal(invt[:, cs], trf[:, cs])
            nc.vector.tensor_tensor(posn[:, cs], pos[:, cs], invt[:, cs],
                                    op=Alu.mult)
            nc.vector.tensor_tensor(ea2s[:, cs], ea2[:, cs], invt[:, cs],
                                    op=Alu.mult)
            # r = terr^2 + ea2s - LAM*posn^2   (LAM baked into WEA2)
            te.tensor_tensor(terr[:, cs], posn[:, cs], xblk[:, cs],
                             op=Alu.subtract)
            te.tensor_tensor(t2[:, cs], terr[:, cs], terr[:, cs], op=Alu.mult)
            if hf == 0:
                nc.vector.scalar_tensor_tensor(np2[:, cs], posn[:, cs], -LAM,
                                               posn[:, cs],
                                               op0=Alu.mult, op1=Alu.mult)
            else:
                nc.gpsimd.tensor_tensor(np2[:, cs], posn[:, cs], posn[:, cs],
                                        op=Alu.mult)
                nc.gpsimd.tensor_tensor(np2[:, cs], np2[:, cs],
                                        lamw[:, :], op=Alu.mult)
            te.tensor_tensor(r1[:, cs], t2[:, cs], ea2s[:, cs], op=Alu.add)
            te.tensor_tensor(r[:, cs], r1[:, cs], np2[:, cs], op=Alu.add)
            nc.vector.tensor_reduce(outv[:, hf:hf+1], r[:, cs],
                                    axis=mybir.AxisListType.X, op=Alu.add)
            nc.sync.dma_start(d_out[:, hf:hf+1], outv[:, hf:hf+1])
    nc.compile()
    return nc


def kernel(A_real, A_imag, X):
    from concourse.bass_utils import run_bass_kernel_spmd

    per_core = _build_host_tensors(
        np.asarray(A_real, np.float32), np.asarray(A_imag, np.float32),
        np.asarray(X, np.float32))

    if "nc" not in _prog_cache:
        _prog_cache["nc"] = build_program()
    nc = _prog_cache["nc"]

    in_maps = [per_core[c] for c in range(NCORES)]
    res = run_bass_kernel_spmd(nc, in_maps, list(range(NCORES)))
    total = 0.0
    for c in range(NCORES):
        total += float(np.asarray(res.results[c]["out"], np.float64).sum())
    loss = total / N
    return np.float32(loss)


# revision 76
# speedup vs baseline: 1.0163x; 1.0163x over previous
"""Trainium2 Bass kernel for nn_EnergyLoss: batched 16x16 complex Hermitian
ground-state projector via shifted matrix-squaring power iteration.

Math summary (all derived from the reference):
  H[n] = 0.5*G - 0.5*sum_d X[n,d]*S_d + (0.5*q_n + EPS)*I,
     G = sum_d A_d A_d^H,  S_d = A_d + A_d^H,  q_n = sum_d X[n,d]^2
  B0 = I - H/(C_SHIFT*||H||_F)   (C_SHIFT tuned so f > (lmax+l1)/2 for every
     sample => |mu(l0)| dominates and the iteration converges to the ground
     state; c<1 tightens the spectral gap and cuts squaring steps)
  B <- B^2, renormalized by tr(B^2)=||B||_F^2 on steps E={1,3,5,7}; the
     normalizer is computed from the state one step EARLIER (on the idle
     N-steps) so the E-step cast never waits on a reduce.
  loss terms from the projector via rowsums (see WPOS/WEA2).
Complex 16x16 matrices are embedded as real 32x32 M(B)=[[Br,-Bi],[Bi,Br]];
per-sample squaring runs as 32x32 PE-array tile matmuls (4 samples per 128
partitions, diagonal tile_positions).  State is fp16, PSUM fp32.

Per-sample 1/(C_SHIFT*fro(H)) is exact on host via the quadratic form
fro^2 = y^T (WH WH^T) y, y=[x;1;q], folded into the XTH columns, so the
device H-matmul directly yields H/f and phase 1 needs no norm pipeline.
Engine split: PE matmuls; DVE psum casts/reduces/shuffles; Act sign-scaled
copies (per-partition scale); Pool state squares + SBUF-only finish ops
(gpsimd cannot touch PSUM or use scalar-broadcast ops on trn2).
"""

import numpy as np

N, D, DIM = 4096, 32, 16
NCORES = 8
NS = N // NCORES          # 512 samples per core
NQ = NS // 4              # 128 quads (4 samples stacked per 128 partitions)
EPS = 1e-5
LAM = 0.1
C_SHIFT = 0.28
KSTEPS = 8
NSLAB = 4                 # quad slabs for pipelining
QS = NQ // NSLAB          # 32 quads per slab

_prog_cache = {}

# packed constant-input byte offsets (per partition), ordered by DMA urgency:
# DMA1 = signs+XTH+WH (H matmuls + first casts), DMA2 = DIAGP (phase-1 STT),
# DMA3 = the rest (first needed by the k=0 normalizer prefetch)
OFF_SIGNP = 0         # f32 [128,1]    4B
OFF_SIGNPM = 4        # f32 [128,1]    4B
OFF_XTH = 16          # f16 [34,512]   1024B  (columns pre-scaled by 1/f_n)
OFF_WH = 1040         # f16 [34,512]   1024B
OFF_DIAGP = 2064      # f16 [128,2048] 4096B
OFF_MASKBS = 6160     # f16 [128,128]  256B   signed block mask (+-1 rows)
OFF_SIGNW = 6416      # f16 [128,512]  1024B  per-partition sign, 512 wide
OFF_XBLK = 7440       # f32 [128,128]  512B
OFF_WPOS = 7952       # f16 [128,128]  256B
OFF_WEA2 = 8208       # f16 [128,128]  256B   (pre-scaled by LAMBDA_REG)
OFF_WTRC = 8464       # f16 [128,16*128] 4096B  per-j trace-pick block masks
OFF_MASKBSF = 12560   # f32 [128,128]  512B   signed mask, f32 for DVE path
OFF_WPOSF = 13072     # f32 [128,128]  512B
OFF_WEA2F = 13584     # f32 [128,128]  512B
CIN_BYTES = 14096
DMA1_HI = OFF_DIAGP
DMA2_HI = OFF_MASKBS
HSWAP = list(range(16, 32)) + list(range(0, 16))


def _build_host_tensors(A_real, A_imag, X):
    """All small A-derived tensors + per-core X-derived layouts (numpy fp32)."""
    A = (A_real + 1j * A_imag).astype(np.complex64)
    Sc = A + np.conj(np.transpose(A, (0, 2, 1)))        # [D,16,16] Hermitian
    Sr, Si = Sc.real.astype(np.float64), Sc.imag.astype(np.float64)
    G = np.einsum('dij,dkj->ik', A, A.conj())
    Gr, Gi = G.real.astype(np.float64), G.imag.astype(np.float64)
    cA = A.sum(axis=1)                                   # [D,16] colsum over i
    cA2 = (A @ A).sum(axis=1)

    # H-build weights: WH[k, 32j+m], contraction k: 0..31 = d, 32 = const, 33 = q
    WH = np.zeros((34, 512), np.float64)
    for j in range(DIM):
        c = 32 * j
        WH[:D, c:c+16] = -0.5 * Sr[:, :, j]              # m<16 -> Hr[m,j]
        WH[:D, c+16:c+32] = -0.5 * Si[:, :, j]           # m>=16 -> Hi[m-16,j]
        WH[32, c:c+16] = 0.5 * Gr[:, j]
        WH[32, c+j] += EPS
        WH[32, c+16:c+32] = 0.5 * Gi[:, j]
        WH[33, c+j] = 0.5
    MQF = WH @ WH.T                                      # fro^2 quadratic form
    # diag delta pattern on the state layout (top halves only)
    DIAGP = np.zeros((128, 16 * NQ), np.float32)
    for s in range(4):
        for i in range(DIM):
            DIAGP[32*s + i, i::16] = 1.0
    # block masks for cross-partition per-sample sums; signed variant bakes
    # the state's [+ / -] partition sign into the broadcast normalizer.
    # WTRC[p, 128j+p'] picks the j-th diagonal row of each block (trace).
    MASKBS = np.zeros((128, 128), np.float32)
    WTRC = np.zeros((128, 16 * 128), np.float32)
    SIGNP = np.ones((128, 1), np.float32)
    for s in range(4):
        SIGNP[32*s+16:32*s+32, 0] = -1.0
    for b in range(4):
        MASKBS[32*b:32*b+32, 32*b:32*b+32] = SIGNP[32*b:32*b+32, 0][None, :]
        for j in range(DIM):
            WTRC[32*b+j, 128*j+32*b:128*j+32*b+32] = 1.0
    # finish functionals: rhs is RS from s2 = [Pr; -Pi] rowsums
    #   pos_raw[32s+d, q] = sum_i cAr[d,i]*rr[i] - cAi[d,i]*ri[i]
    #   RS bottom rows hold -ri  =>  bottom weight = +cAi
    WPOS = np.zeros((128, 128), np.float32)
    WEA2 = np.zeros((128, 128), np.float32)
    for s in range(4):
        b = 32 * s
        WPOS[b:b+16, b:b+32] = cA.real.T                 # [i, d]
        WPOS[b+16:b+32, b:b+32] = cA.imag.T
        WEA2[b:b+16, b:b+32] = LAM * cA2.real.T
        WEA2[b+16:b+32, b:b+32] = LAM * cA2.imag.T

    def put(buf, rows, off, arr):
        b = np.ascontiguousarray(arr).view(np.uint8).reshape(arr.shape[0], -1)
        buf[:rows, off:off+b.shape[1]] = b

    per_core = []
    for c in range(NCORES):
        Xc = np.asarray(X[c*NS:(c+1)*NS], np.float64)    # [512, 32]
        q = (Xc ** 2).sum(1)
        Y = np.concatenate([Xc, np.ones((NS, 1)), q[:, None]], axis=1)  # [512,34]
        fro = np.sqrt(np.einsum('nk,kl,nl->n', Y, MQF, Y))
        invf = 1.0 / (C_SHIFT * fro)                     # [512]
        YS = (Y * invf[:, None]).astype(np.float32)      # scaled y columns
        XTH = np.zeros((34, 512), np.float32)
        XBLK = np.zeros((128, 128), np.float32)
        for s in range(4):
            idx = np.arange(NQ) * 4 + s                  # n_core(q,s)
            XTH[:, 128*s:128*s+128] = YS[idx].T
            XBLK[32*s:32*s+32, :] = Xc[idx].T
        buf = np.zeros((128, CIN_BYTES), np.uint8)
        put(buf, 128, OFF_XBLK, XBLK)
        put(buf, 128, OFF_MASKBS, MASKBS.astype(np.float16))
        put(buf, 128, OFF_SIGNW, np.repeat(SIGNP, 512, 1).astype(np.float16))
        put(buf, 128, OFF_SIGNP, SIGNP)
        put(buf, 128, OFF_SIGNPM, -SIGNP)
        put(buf, 34, OFF_XTH, XTH.astype(np.float16))
        put(buf, 34, OFF_WH, WH.astype(np.float16))
        put(buf, 128, OFF_WPOS, WPOS.astype(np.float16))
        put(buf, 128, OFF_WEA2, WEA2.astype(np.float16))
        put(buf, 128, OFF_WTRC, WTRC.astype(np.float16))
        put(buf, 128, OFF_MASKBSF, MASKBS)
        put(buf, 128, OFF_WPOSF, WPOS)
        put(buf, 128, OFF_WEA2F, WEA2)
        put(buf, 128, OFF_DIAGP, DIAGP.astype(np.float16))
        per_core.append({"cin": buf})
    return per_core


def build_program(ksteps=KSTEPS, warmup=20, PREF_DVE_SLABS=1, WB_PRIO=0):
    import concourse.bass as bass
    import concourse.bacc as bacc
    import concourse.mybir as mybir
    import concourse.tile as tile
    from contextlib import ExitStack

    f16, f32 = mybir.dt.float16, mybir.dt.float32
    u8, u32 = mybir.dt.uint8, mybir.dt.uint32
    Alu = mybir.AluOpType
    Act = mybir.ActivationFunctionType

    nc = bacc.Bacc()
    d_cin = nc.dram_tensor("cin", [128, CIN_BYTES], u8, kind="ExternalInput")
    d_out = nc.dram_tensor("out", [128, 2], f32, kind="ExternalOutput")

    SW = 16 * QS              # state cols per slab (512)

    with tile.TileContext(nc) as tc, ExitStack() as ctx:
        cpool = ctx.enter_context(tc.tile_pool(name="consts", bufs=1))
        spool = ctx.enter_context(tc.tile_pool(name="state", bufs=3))
        wpool = ctx.enter_context(tc.tile_pool(name="work", bufs=2))
        ppool_pm = ctx.enter_context(tc.tile_pool(name="psum_pm", bufs=6, space="PSUM"))
        ppool_sm = ctx.enter_context(tc.tile_pool(name="psum_sm", bufs=2, space="PSUM"))

        # ---------------- input DMAs (critical slice first) -----------------
        cst = cpool.tile([128, CIN_BYTES], u8, tag="cin")
        nc.sync.dma_start(cst[:, 0:DMA1_HI], d_cin[:, 0:DMA1_HI])
        nc.sync.dma_start(cst[:, DMA1_HI:DMA1_HI+1024],
                          d_cin[:, DMA1_HI:DMA1_HI+1024])
        nc.sync.dma_start(cst[:, DMA1_HI+1024:DMA2_HI],
                          d_cin[:, DMA1_HI+1024:DMA2_HI])
        nc.sync.dma_start(cst[:, DMA2_HI:], d_cin[:, DMA2_HI:])
        xblk = cst[:, OFF_XBLK:OFF_XBLK+512].bitcast(f32)
        maskbs = cst[:, OFF_MASKBS:OFF_MASKBS+256].bitcast(f16)
        signw = cst[:, OFF_SIGNW:OFF_SIGNW+1024].bitcast(f16)
        signp = cst[:, OFF_SIGNP:OFF_SIGNP+4].bitcast(f32)
        signpm = cst[:, OFF_SIGNPM:OFF_SIGNPM+4].bitcast(f32)
        xth = cst[:, OFF_XTH:OFF_XTH+1024].bitcast(f16)[0:34, :]
        wh = cst[:, OFF_WH:OFF_WH+1024].bitcast(f16)[0:34, :]
        wpos = cst[:, OFF_WPOS:OFF_WPOS+256].bitcast(f16)
        wea2 = cst[:, OFF_WEA2:OFF_WEA2+256].bitcast(f16)
        wtrc = cst[:, OFF_WTRC:OFF_WTRC+4096].bitcast(f16)
        maskbsf = cst[:, OFF_MASKBSF:OFF_MASKBSF+512].bitcast(f32)
        wposf = cst[:, OFF_WPOSF:OFF_WPOSF+512].bitcast(f32)
        wea2f = cst[:, OFF_WEA2F:OFF_WEA2F+512].bitcast(f32)
        diagp = cst[:, OFF_DIAGP:OFF_DIAGP+4096].bitcast(f16)

        # ---------------- PE p-state warmup during the DMA ------------------
        if warmup:
            wz = wpool.tile([128, 128], f16, tag="warm")
            nc.gpsimd.memset(wz[:, :], 0)
            wz2 = wpool.tile([128, 128], f16, tag="warm2")
            nc.scalar.activation(wz2[:, :], wz[:, :], Act.Copy)
            pwarm = ppool_pm.tile([128, 512], f32, tag="pm")
            for i in range(warmup):
                nc.tensor.matmul(pwarm[:, 128*(i % 4):128*(i % 4)+128],
                                 wz[:, :], wz[:, :], start=True, stop=True)

        # ---------------- phase 1: H/f build, B0 = I - H/f ------------------
        # Quad-sliced by slab so slab 0's state (and its wb + step-0 matmuls)
        # flows while slabs 1-3 are still streaming H.
        s2 = spool.tile([128, 2048], f16, tag="s2")
        p1t = wpool.tile([128, 2048], f16, tag="p1t")
        for sl in range(NSLAB):
            ph = ppool_pm.tile([128, SW], f32, tag="pm")
            for j in range(DIM):
                for s in range(4):
                    nc.tensor.matmul(
                        ph[32*s:32*s+32, QS*j:QS*(j+1)],
                        wh[:, 32*j:32*j+32],
                        xth[:, 128*s+QS*sl:128*s+QS*(sl+1)],
                        start=True, stop=True,
                        tile_position=(0, 32*s),
                    )
            # s2 slab = diagp + pm*signpm: Act does the psum read + sign,
            # DVE adds the diagonal (pipelines across slabs)
            nc.scalar.activation(
                p1t[:, SW*sl:SW*(sl+1)].rearrange("p (q j) -> p q j", j=DIM),
                ph[:, :].rearrange("p (j q) -> p q j", j=DIM),
                Act.Copy, scale=signpm[:, :])
            nc.vector.tensor_tensor(s2[:, SW*sl:SW*(sl+1)],
                                    p1t[:, SW*sl:SW*(sl+1)],
                                    diagp[:, SW*sl:SW*(sl+1)], op=Alu.add)

        def build_wb_left(wb_t, s2_t, sl):
            """wb[:, 32q+0:16] = s2*signp (-> [Br;Bi]).  Alternate slabs go to
            Act (per-partition scale) and Pool (real-tensor multiply by the
            materialized sign plane -- Pool can't use scalar-broadcast ops)."""
            wbl = wb_t[:, :].rearrange("p (q j) -> p q j", j=32)
            src = s2_t[:, SW*sl:SW*(sl+1)].rearrange("p (q j) -> p q j", j=DIM)
            dst = wbl[:, sl*QS:(sl+1)*QS, 0:16]
            if sl % 2 == 0:
                nc.scalar.activation(dst, src, Act.Copy, scale=signp[:, :])
            else:
                nc.gpsimd.tensor_tensor(
                    dst, src,
                    signw[:, 0:SW].rearrange("p (q j) -> p q j", j=DIM),
                    op=Alu.mult)

        def build_wb_right(wb_t, s2_t, sl):
            """wb[:, 32q+16:32] = partition-half-swapped s2 (-> [-Bi;Br])."""
            wbw = wb_t[:, :].bitcast(u32).rearrange("p (q w) -> p q w", w=16)
            s2w = s2_t[:, :].bitcast(u32)
            nc.vector.stream_shuffle(
                wbw[:, sl*QS:(sl+1)*QS, 8:16],
                s2w[:, 8*sl*QS:8*(sl+1)*QS].rearrange("p (q w) -> p q w", w=8),
                mask=HSWAP)

        scl = [None, None]    # per slab-pair: [128, 64] tiles of +-1/s

        def norm_prefetch(s2_t, sl, sq_t, trp_t):
            """Normalizer for the NEXT step's cast: fro^2 of the state slab.
            Square on Pool (SBUF only); the (partition-block x j) double sum
            runs as 16 PSUM-accumulating matmuls over j-strided rhs slices
            with the sign-baked mask weights -- zero DVE cost; one batched
            recip per slab-pair then yields +-1/s."""
            c0, c1 = SW*sl, SW*(sl+1)
            # the normalizer only controls fp16 range, so sample EVEN
            # j-columns only: half the square/reduce volume, same behavior
            JH2 = DIM // 2
            sc0 = JH2 * QS * sl
            seven = s2_t[:, c0:c1].rearrange(
                "p (q j two) -> p q j two", j=JH2, two=2)[:, :, :, 0]
            sqc = sq_t[:, sc0:sc0+JH2*QS].rearrange("p (q j) -> p q j", j=JH2)
            nc.vector.tensor_tensor(sqc, seven, seven, op=Alu.mult)
            slot = sl % 2
            if sl >= PREF_DVE_SLABS:
                sqj = sq_t[:, sc0:sc0+JH2*QS].rearrange(
                    "p (q j) -> p j q", j=JH2)
                for j in range(JH2):
                    nc.tensor.matmul(trp_t[:, QS*slot:QS*(slot+1)],
                                     maskbs[:, :], sqj[:, j, :],
                                     start=(j == 0), stop=(j == JH2 - 1))
            else:
                pr = wpool.tile([128, QS], f32, tag=f"pr{sl}")
                nc.vector.tensor_reduce(
                    pr[:, :],
                    sq_t[:, sc0:sc0+JH2*QS].rearrange("p (q j) -> p q j",
                                                      j=JH2),
                    axis=mybir.AxisListType.X, op=Alu.add)
                nc.tensor.matmul(trp_t[:, QS*slot:QS*(slot+1)], maskbsf[:, :],
                                 pr[:, :], start=True, stop=True)
            if slot == 1:
                # recip per slab-pair: ready mid-step, so next step's E-casts
                # for these slabs never wait on the tail slab's prefetch
                hp = sl // 2
                scl_t = wpool.tile([128, 2*QS], f32, tag=f"scl{hp}")
                nc.vector.reciprocal(scl_t[:, :], trp_t[:, :])
                scl[hp] = scl_t

        wb = spool.tile([128, 4096], f16, tag="wb")
        for sl in range(NSLAB):
            if sl % 2 == 0:
                build_wb_left(wb, s2, sl)
            else:
                wbl = wb[:, :].rearrange("p (q j) -> p q j", j=32)
                nc.vector.tensor_scalar_mul(
                    wbl[:, sl*QS:(sl+1)*QS, 0:16],
                    s2[:, SW*sl:SW*(sl+1)].rearrange("p (q j) -> p q j", j=DIM),
                    signp[:, :])
            build_wb_right(wb, s2, sl)

        # ---------------- phase 3: squaring iteration -----------------------
        # E-steps (normalized cast) on odd k; normalizer prefetched on even k.
        for k in range(ksteps):
            last = (k == ksteps - 1)
            # E-steps are STAGGERED by slab-pair (slabs 0-1 normalize on odd
            # steps, slabs 2-3 on even) so every step carries a uniform
            # DVE-cast/Act-cast/prefetch mix; the last step is unnormalized
            # for all slabs (the trace division absorbs the scale)
            s2n = spool.tile([128, 2048], f16, tag="s2")
            wbn = None if last else spool.tile([128, 4096], f16, tag="wb")
            sqn = trpn = None
            if k < ksteps - 2:
                sqn = wpool.tile([128, 1024], f16, tag="sq", name="sqn")
                trpn = ppool_sm.tile([128, 2*QS], f32, tag="sm", name="trpn")
            for sl in range(NSLAB):
                exact = (not last) and k >= 1 and \
                    (k % 2 == (1 if sl < 2 else 0))
                pref = (not last) and k + 1 < ksteps - 1 and \
                    ((k + 1) % 2 == (1 if sl < 2 else 0))
                q0 = sl * QS
                pm = ppool_pm.tile([128, SW], f32, tag="pm")
                for qq in range(QS):
                    q = q0 + qq
                    for s in range(4):
                        nc.tensor.matmul(
                            pm[32*s:32*s+32, 16*qq:16*qq+16],
                            wb[32*s:32*s+32, 32*q:32*q+32],
                            wb[32*s:32*s+32, 32*q:32*q+16],
                            start=True, stop=True,
                            tile_position=(32*s, 32*s))
                dst = s2n[:, SW*sl:SW*(sl+1)].rearrange("p (q j) -> p q j", j=DIM)
                src = pm[:, :].rearrange("p (q j) -> p q j", j=DIM)
                if exact:
                    # cast: s2' = pm * (+-1/s), per-quad scale -> DVE TT
                    nc.vector.tensor_tensor(
                        dst, src,
                        scl[sl // 2][:, QS*(sl % 2):QS*(sl % 2 + 1)]
                            .unsqueeze(-1).broadcast_to([128, QS, DIM]),
                        op=Alu.mult)
                elif last and sl % 2 == 1:
                    # last step: DVE is idle, split the casts with Act
                    nc.vector.tensor_scalar_mul(dst, src, signp[:, :])
                else:
                    # cast: s2' = pm * sign (no normalization this step)
                    nc.scalar.activation(dst, src, Act.Copy, scale=signp[:, :])
                if not last:
                    with tc.high_priority(offset=WB_PRIO):
                        build_wb_left(wbn, s2n, sl)
                        build_wb_right(wbn, s2n, sl)
                    if pref:
                        norm_prefetch(s2n, sl, sqn, trpn)
            s2 = s2n
            if not last:
                wb = wbn

        # ---------------- phase 4: finish (per-slab for pipelining) ---------
        # pos/ea2/trace fold their j-rowsums into PSUM-accumulating matmuls
        # over j-strided state slices: no DVE reduces at all in the tail
        trf = ppool_sm.tile([128, 128], f32, tag="sm")
        pos = ppool_sm.tile([128, 128], f32, tag="sm")
        ea2 = ppool_sm.tile([128, 128], f32, tag="sm")
        def s2j(sl):
            return s2[:, SW*sl:SW*(sl+1)].rearrange("p (q j) -> p j q", j=DIM)
        for w_t, out_t in ((wtrc, trf), (wpos, pos), (wea2, ea2)):
            for sl in range(NSLAB):
                qs = slice(QS*sl, QS*(sl+1))
                for j in range(DIM):
                    wsl = w_t[:, 128*j:128*(j+1)] if w_t is wtrc else w_t[:, :]
                    nc.tensor.matmul(out_t[:, qs], wsl, s2j(sl)[:, j, :],
                                     start=(j == 0), stop=(j == DIM - 1))
        # the rest is column-sliceable; per-slab chains run concurrently on
        # DVE and Pool (PSUM readers stay on DVE; Pool's chain avoids scalar
        # ops -- TensorScalarPtr is illegal there -- via LAMW); the output
        # DMA is split so the first half's latency hides under the second
        invt = wpool.tile([128, 128], f32, tag="invt")
        posn = wpool.tile([128, 128], f32, tag="posn")
        ea2s = wpool.tile([128, 128], f32, tag="ea2s")
        terr = wpool.tile([128, 128], f32, tag="terr")
        t2 = wpool.tile([128, 128], f32, tag="t2")
        np2 = wpool.tile([128, 128], f32, tag="np2")
        r1 = wpool.tile([128, 128], f32, tag="r1")
        r = wpool.tile([128, 128], f32, tag="r")
        outv = wpool.tile([128, 2], f32, tag="outv")
        lamw = wpool.tile([128, 64], f32, tag="lamw")
        nc.gpsimd.memset(lamw[:, :], -LAM)
        for hf in range(2):
            cs = slice(64*hf, 64*hf+64)
            te = nc.vector if hf == 0 else nc.gpsimd
            nc.vector.reciprocal(invt[:, cs], trf[:, cs])
            nc.vector.tensor_tensor(posn[:, cs], pos[:, cs], invt[:, cs],
                                    op=Alu.mult)
            nc.vector.tensor_tensor(ea2s[:, cs], ea2[:, cs], invt[:, cs],
                                    op=Alu.mult)
            # r = terr^2 + ea2s - LAM*posn^2   (LAM baked into WEA2)
            te.tensor_tensor(terr[:, cs], posn[:, cs], xblk[:, cs],
                             op=Alu.subtract)
            te.tensor_tensor(t2[:, cs], terr[:, cs], terr[:, cs], op=Alu.mult)
            if hf == 0:
                nc.vector.scalar_tensor_tensor(np2[:, cs], posn[:, cs], -LAM,
                                               posn[:, cs],
                                               op0=Alu.mult, op1=Alu.mult)
            else:
                nc.gpsimd.tensor_tensor(np2[:, cs], posn[:, cs], posn[:, cs],
                                        op=Alu.mult)
                nc.gpsimd.tensor_tensor(np2[:, cs], np2[:, cs],
                                        lamw[:, :], op=Alu.mult)
            te.tensor_tensor(r1[:, cs], t2[:, cs], ea2s[:, cs], op=Alu.add)
            te.tensor_tensor(r[:, cs], r1[:, cs], np2[:, cs], op=Alu.add)
            nc.vector.tensor_reduce(outv[:, hf:hf+1], r[:, cs],
                                    axis=mybir.AxisListType.X, op=Alu.add)
            nc.sync.dma_start(d_out[:, hf:hf+1], outv[:, hf:hf+1])
    nc.compile()
    return nc


def kernel(A_real, A_imag, X):
    from concourse.bass_utils import run_bass_kernel_spmd

    per_core = _build_host_tensors(
        np.asarray(A_real, np.float32), np.asarray(A_imag, np.float32),
        np.asarray(X, np.float32))

    if "nc" not in _prog_cache:
        _prog_cache["nc"] = build_program()
    nc = _prog_cache["nc"]

    in_maps = [per_core[c] for c in range(NCORES)]
    res = run_bass_kernel_spmd(nc, in_maps, list(range(NCORES)))
    total = 0.0
    for c in range(NCORES):
        total += float(np.asarray(res.results[c]["out"], np.float64).sum())
    loss = total / N
    return np.float32(loss)


# revision 77
# speedup vs baseline: 1.0276x; 1.0111x over previous
"""Trainium2 Bass kernel for nn_EnergyLoss: batched 16x16 complex Hermitian
ground-state projector via shifted matrix-squaring power iteration.

Math summary (all derived from the reference):
  H[n] = 0.5*G - 0.5*sum_d X[n,d]*S_d + (0.5*q_n + EPS)*I,
     G = sum_d A_d A_d^H,  S_d = A_d + A_d^H,  q_n = sum_d X[n,d]^2
  B0 = I - H/(C_SHIFT*||H||_F)   (C_SHIFT tuned so f > (lmax+l1)/2 for every
     sample => |mu(l0)| dominates and the iteration converges to the ground
     state; c<1 tightens the spectral gap and cuts squaring steps)
  B <- B^2, renormalized by tr(B^2)=||B||_F^2 on steps E={1,3,5,7}; the
     normalizer is computed from the state one step EARLIER (on the idle
     N-steps) so the E-step cast never waits on a reduce.
  loss terms from the projector via rowsums (see WPOS/WEA2).
Complex 16x16 matrices are embedded as real 32x32 M(B)=[[Br,-Bi],[Bi,Br]];
per-sample squaring runs as 32x32 PE-array tile matmuls (4 samples per 128
partitions, diagonal tile_positions).  State is fp16, PSUM fp32.

Per-sample 1/(C_SHIFT*fro(H)) is exact on host via the quadratic form
fro^2 = y^T (WH WH^T) y, y=[x;1;q], folded into the XTH columns, so the
device H-matmul directly yields H/f and phase 1 needs no norm pipeline.
Engine split: PE matmuls; DVE psum casts/reduces/shuffles; Act sign-scaled
copies (per-partition scale); Pool state squares + SBUF-only finish ops
(gpsimd cannot touch PSUM or use scalar-broadcast ops on trn2).
"""

import numpy as np

N, D, DIM = 4096, 32, 16
NCORES = 8
NS = N // NCORES          # 512 samples per core
NQ = NS // 4              # 128 quads (4 samples stacked per 128 partitions)
EPS = 1e-5
LAM = 0.1
C_SHIFT = 0.28
KSTEPS = 8
NSLAB = 4                 # quad slabs for pipelining
QS = NQ // NSLAB          # 32 quads per slab

_prog_cache = {}

# packed constant-input byte offsets (per partition), ordered by DMA urgency:
# DMA1 = signs+XTH+WH (H matmuls + first casts), DMA2 = DIAGP (phase-1 STT),
# DMA3 = the rest (first needed by the k=0 normalizer prefetch)
OFF_SIGNP = 0         # f32 [128,1]    4B
OFF_SIGNPM = 4        # f32 [128,1]    4B
OFF_XTH = 16          # f16 [34,512]   1024B  (columns pre-scaled by 1/f_n)
OFF_WH = 1040         # f16 [34,512]   1024B
OFF_DIAGP = 2064      # f16 [128,2048] 4096B
OFF_MASKBS = 6160     # f16 [128,128]  256B   signed block mask (+-1 rows)
OFF_SIGNW = 6416      # f16 [128,512]  1024B  per-partition sign, 512 wide
OFF_XBLK = 7440       # f32 [128,128]  512B
OFF_WPOS = 7952       # f16 [128,128]  256B
OFF_WEA2 = 8208       # f16 [128,128]  256B   (pre-scaled by LAMBDA_REG)
OFF_WTRC = 8464       # f16 [128,16*128] 4096B  per-j trace-pick block masks
OFF_MASKBSF = 12560   # f32 [128,128]  512B   signed mask, f32 for DVE path
OFF_WPOSF = 13072     # f32 [128,128]  512B
OFF_WEA2F = 13584     # f32 [128,128]  512B
CIN_BYTES = 14096
DMA1_HI = OFF_DIAGP
DMA2_HI = OFF_MASKBS
HSWAP = list(range(16, 32)) + list(range(0, 16))


def _build_host_tensors(A_real, A_imag, X):
    """All small A-derived tensors + per-core X-derived layouts (numpy fp32)."""
    A = (A_real + 1j * A_imag).astype(np.complex64)
    Sc = A + np.conj(np.transpose(A, (0, 2, 1)))        # [D,16,16] Hermitian
    Sr, Si = Sc.real.astype(np.float64), Sc.imag.astype(np.float64)
    G = np.einsum('dij,dkj->ik', A, A.conj())
    Gr, Gi = G.real.astype(np.float64), G.imag.astype(np.float64)
    cA = A.sum(axis=1)                                   # [D,16] colsum over i
    cA2 = (A @ A).sum(axis=1)

    # H-build weights: WH[k, 32j+m], contraction k: 0..31 = d, 32 = const, 33 = q
    WH = np.zeros((34, 512), np.float64)
    for j in range(DIM):
        c = 32 * j
        WH[:D, c:c+16] = -0.5 * Sr[:, :, j]              # m<16 -> Hr[m,j]
        WH[:D, c+16:c+32] = -0.5 * Si[:, :, j]           # m>=16 -> Hi[m-16,j]
        WH[32, c:c+16] = 0.5 * Gr[:, j]
        WH[32, c+j] += EPS
        WH[32, c+16:c+32] = 0.5 * Gi[:, j]
        WH[33, c+j] = 0.5
    MQF = WH @ WH.T                                      # fro^2 quadratic form
    # diag delta pattern on the state layout (top halves only)
    DIAGP = np.zeros((128, 16 * NQ), np.float32)
    for s in range(4):
        for i in range(DIM):
            DIAGP[32*s + i, i::16] = 1.0
    # block masks for cross-partition per-sample sums; signed variant bakes
    # the state's [+ / -] partition sign into the broadcast normalizer.
    # WTRC[p, 128j+p'] picks the j-th diagonal row of each block (trace).
    MASKBS = np.zeros((128, 128), np.float32)
    WTRC = np.zeros((128, 16 * 128), np.float32)
    SIGNP = np.ones((128, 1), np.float32)
    for s in range(4):
        SIGNP[32*s+16:32*s+32, 0] = -1.0
    for b in range(4):
        MASKBS[32*b:32*b+32, 32*b:32*b+32] = SIGNP[32*b:32*b+32, 0][None, :]
        for j in range(DIM):
            WTRC[32*b+j, 128*j+32*b:128*j+32*b+32] = 1.0
    # finish functionals: rhs is RS from s2 = [Pr; -Pi] rowsums
    #   pos_raw[32s+d, q] = sum_i cAr[d,i]*rr[i] - cAi[d,i]*ri[i]
    #   RS bottom rows hold -ri  =>  bottom weight = +cAi
    WPOS = np.zeros((128, 128), np.float32)
    WEA2 = np.zeros((128, 128), np.float32)
    for s in range(4):
        b = 32 * s
        WPOS[b:b+16, b:b+32] = cA.real.T                 # [i, d]
        WPOS[b+16:b+32, b:b+32] = cA.imag.T
        WEA2[b:b+16, b:b+32] = LAM * cA2.real.T
        WEA2[b+16:b+32, b:b+32] = LAM * cA2.imag.T

    def put(buf, rows, off, arr):
        b = np.ascontiguousarray(arr).view(np.uint8).reshape(arr.shape[0], -1)
        buf[:rows, off:off+b.shape[1]] = b

    per_core = []
    for c in range(NCORES):
        Xc = np.asarray(X[c*NS:(c+1)*NS], np.float64)    # [512, 32]
        q = (Xc ** 2).sum(1)
        Y = np.concatenate([Xc, np.ones((NS, 1)), q[:, None]], axis=1)  # [512,34]
        fro = np.sqrt(np.einsum('nk,kl,nl->n', Y, MQF, Y))
        invf = 1.0 / (C_SHIFT * fro)                     # [512]
        YS = (Y * invf[:, None]).astype(np.float32)      # scaled y columns
        XTH = np.zeros((34, 512), np.float32)
        XBLK = np.zeros((128, 128), np.float32)
        for s in range(4):
            idx = np.arange(NQ) * 4 + s                  # n_core(q,s)
            XTH[:, 128*s:128*s+128] = YS[idx].T
            XBLK[32*s:32*s+32, :] = Xc[idx].T
        buf = np.zeros((128, CIN_BYTES), np.uint8)
        put(buf, 128, OFF_XBLK, XBLK)
        put(buf, 128, OFF_MASKBS, MASKBS.astype(np.float16))
        put(buf, 128, OFF_SIGNW, np.repeat(SIGNP, 512, 1).astype(np.float16))
        put(buf, 128, OFF_SIGNP, SIGNP)
        put(buf, 128, OFF_SIGNPM, -SIGNP)
        put(buf, 34, OFF_XTH, XTH.astype(np.float16))
        put(buf, 34, OFF_WH, WH.astype(np.float16))
        put(buf, 128, OFF_WPOS, WPOS.astype(np.float16))
        put(buf, 128, OFF_WEA2, WEA2.astype(np.float16))
        put(buf, 128, OFF_WTRC, WTRC.astype(np.float16))
        put(buf, 128, OFF_MASKBSF, MASKBS)
        put(buf, 128, OFF_WPOSF, WPOS)
        put(buf, 128, OFF_WEA2F, WEA2)
        put(buf, 128, OFF_DIAGP, DIAGP.astype(np.float16))
        per_core.append({"cin": buf})
    return per_core


def build_program(ksteps=KSTEPS, warmup=20, PREF_DVE_SLABS=1, WB_PRIO=0):
    import concourse.bass as bass
    import concourse.bacc as bacc
    import concourse.mybir as mybir
    import concourse.tile as tile
    from contextlib import ExitStack

    f16, f32 = mybir.dt.float16, mybir.dt.float32
    u8, u32 = mybir.dt.uint8, mybir.dt.uint32
    Alu = mybir.AluOpType
    Act = mybir.ActivationFunctionType

    nc = bacc.Bacc()
    d_cin = nc.dram_tensor("cin", [128, CIN_BYTES], u8, kind="ExternalInput")
    d_out = nc.dram_tensor("out", [128, 2], f32, kind="ExternalOutput")

    SW = 16 * QS              # state cols per slab (512)

    with tile.TileContext(nc) as tc, ExitStack() as ctx:
        cpool = ctx.enter_context(tc.tile_pool(name="consts", bufs=1))
        spool = ctx.enter_context(tc.tile_pool(name="state", bufs=3))
        wpool = ctx.enter_context(tc.tile_pool(name="work", bufs=2))
        ppool_pm = ctx.enter_context(tc.tile_pool(name="psum_pm", bufs=6, space="PSUM"))
        ppool_sm = ctx.enter_context(tc.tile_pool(name="psum_sm", bufs=2, space="PSUM"))

        # ---------------- input DMAs (critical slice first) -----------------
        cst = cpool.tile([128, CIN_BYTES], u8, tag="cin")
        nc.sync.dma_start(cst[:, 0:DMA1_HI], d_cin[:, 0:DMA1_HI])
        nc.sync.dma_start(cst[:, DMA1_HI:DMA1_HI+1024],
                          d_cin[:, DMA1_HI:DMA1_HI+1024])
        nc.sync.dma_start(cst[:, DMA1_HI+1024:DMA2_HI],
                          d_cin[:, DMA1_HI+1024:DMA2_HI])
        nc.sync.dma_start(cst[:, DMA2_HI:], d_cin[:, DMA2_HI:])
        xblk = cst[:, OFF_XBLK:OFF_XBLK+512].bitcast(f32)
        maskbs = cst[:, OFF_MASKBS:OFF_MASKBS+256].bitcast(f16)
        signw = cst[:, OFF_SIGNW:OFF_SIGNW+1024].bitcast(f16)
        signp = cst[:, OFF_SIGNP:OFF_SIGNP+4].bitcast(f32)
        signpm = cst[:, OFF_SIGNPM:OFF_SIGNPM+4].bitcast(f32)
        xth = cst[:, OFF_XTH:OFF_XTH+1024].bitcast(f16)[0:34, :]
        wh = cst[:, OFF_WH:OFF_WH+1024].bitcast(f16)[0:34, :]
        wpos = cst[:, OFF_WPOS:OFF_WPOS+256].bitcast(f16)
        wea2 = cst[:, OFF_WEA2:OFF_WEA2+256].bitcast(f16)
        wtrc = cst[:, OFF_WTRC:OFF_WTRC+4096].bitcast(f16)
        maskbsf = cst[:, OFF_MASKBSF:OFF_MASKBSF+512].bitcast(f32)
        wposf = cst[:, OFF_WPOSF:OFF_WPOSF+512].bitcast(f32)
        wea2f = cst[:, OFF_WEA2F:OFF_WEA2F+512].bitcast(f32)
        diagp = cst[:, OFF_DIAGP:OFF_DIAGP+4096].bitcast(f16)

        # ---------------- PE p-state warmup during the DMA ------------------
        if warmup:
            wz = wpool.tile([128, 128], f16, tag="warm")
            nc.gpsimd.memset(wz[:, :], 0)
            wz2 = wpool.tile([128, 128], f16, tag="warm2")
            nc.scalar.activation(wz2[:, :], wz[:, :], Act.Copy)
            pwarm = ppool_pm.tile([128, 512], f32, tag="pm")
            for i in range(warmup):
                nc.tensor.matmul(pwarm[:, 128*(i % 4):128*(i % 4)+128],
                                 wz[:, :], wz[:, :], start=True, stop=True)

        # ---------------- phase 1: H/f build, B0 = I - H/f ------------------
        # Quad-sliced by slab so slab 0's state (and its wb + step-0 matmuls)
        # flows while slabs 1-3 are still streaming H.
        s2 = spool.tile([128, 2048], f16, tag="s2")
        p1t = wpool.tile([128, 2048], f16, tag="p1t")
        for sl in range(NSLAB):
            ph = ppool_pm.tile([128, SW], f32, tag="pm")
            for j in range(DIM):
                for s in range(4):
                    nc.tensor.matmul(
                        ph[32*s:32*s+32, QS*j:QS*(j+1)],
                        wh[:, 32*j:32*j+32],
                        xth[:, 128*s+QS*sl:128*s+QS*(sl+1)],
                        start=True, stop=True,
                        tile_position=(0, 32*s),
                    )
            # s2 slab = diagp + pm*signpm: Act does the psum read + sign,
            # DVE adds the diagonal (pipelines across slabs)
            nc.scalar.activation(
                p1t[:, SW*sl:SW*(sl+1)].rearrange("p (q j) -> p q j", j=DIM),
                ph[:, :].rearrange("p (j q) -> p q j", j=DIM),
                Act.Copy, scale=signpm[:, :])
            nc.vector.tensor_tensor(s2[:, SW*sl:SW*(sl+1)],
                                    p1t[:, SW*sl:SW*(sl+1)],
                                    diagp[:, SW*sl:SW*(sl+1)], op=Alu.add)

        def build_wb_left(wb_t, s2_t, sl):
            """wb[:, 32q+0:16] = s2*signp (-> [Br;Bi]).  Alternate slabs go to
            Act (per-partition scale) and Pool (real-tensor multiply by the
            materialized sign plane -- Pool can't use scalar-broadcast ops)."""
            wbl = wb_t[:, :].rearrange("p (q j) -> p q j", j=32)
            src = s2_t[:, SW*sl:SW*(sl+1)].rearrange("p (q j) -> p q j", j=DIM)
            dst = wbl[:, sl*QS:(sl+1)*QS, 0:16]
            if sl % 2 == 0:
                nc.scalar.activation(dst, src, Act.Copy, scale=signp[:, :])
            else:
                nc.gpsimd.tensor_tensor(
                    dst, src,
                    signw[:, 0:SW].rearrange("p (q j) -> p q j", j=DIM),
                    op=Alu.mult)

        def build_wb_right(wb_t, s2_t, sl):
            """wb[:, 32q+16:32] = partition-half-swapped s2 (-> [-Bi;Br])."""
            wbw = wb_t[:, :].bitcast(u32).rearrange("p (q w) -> p q w", w=16)
            s2w = s2_t[:, :].bitcast(u32)
            nc.vector.stream_shuffle(
                wbw[:, sl*QS:(sl+1)*QS, 8:16],
                s2w[:, 8*sl*QS:8*(sl+1)*QS].rearrange("p (q w) -> p q w", w=8),
                mask=HSWAP)

        scl = [None, None]    # per slab-pair: [128, 64] tiles of +-1/s

        def norm_prefetch(s2_t, sl, sq_t, trp_t):
            """Normalizer for the NEXT step's cast: fro^2 of the state slab.
            Square on Pool (SBUF only); the (partition-block x j) double sum
            runs as 16 PSUM-accumulating matmuls over j-strided rhs slices
            with the sign-baked mask weights -- zero DVE cost; one batched
            recip per slab-pair then yields +-1/s."""
            c0, c1 = SW*sl, SW*(sl+1)
            # the normalizer only controls fp16 range, so sample every
            # FOURTH j-column: quarter the square/block-sum volume (stride 8
            # overflows fp16 on the even-E schedule; 4 holds with margin)
            JH2 = DIM // 4
            sc0 = JH2 * QS * sl
            seven = s2_t[:, c0:c1].rearrange(
                "p (q j four) -> p q j four", j=JH2, four=4)[:, :, :, 0]
            sqc = sq_t[:, sc0:sc0+JH2*QS].rearrange("p (q j) -> p q j", j=JH2)
            nc.vector.tensor_tensor(sqc, seven, seven, op=Alu.mult)
            slot = sl % 2
            if sl >= PREF_DVE_SLABS:
                sqj = sq_t[:, sc0:sc0+JH2*QS].rearrange(
                    "p (q j) -> p j q", j=JH2)
                for j in range(JH2):
                    nc.tensor.matmul(trp_t[:, QS*slot:QS*(slot+1)],
                                     maskbs[:, :], sqj[:, j, :],
                                     start=(j == 0), stop=(j == JH2 - 1))
            else:
                pr = wpool.tile([128, QS], f32, tag=f"pr{sl}")
                nc.vector.tensor_reduce(
                    pr[:, :],
                    sq_t[:, sc0:sc0+JH2*QS].rearrange("p (q j) -> p q j",
                                                      j=JH2),
                    axis=mybir.AxisListType.X, op=Alu.add)
                nc.tensor.matmul(trp_t[:, QS*slot:QS*(slot+1)], maskbsf[:, :],
                                 pr[:, :], start=True, stop=True)
            if slot == 1:
                # recip per slab-pair: ready mid-step, so next step's E-casts
                # for these slabs never wait on the tail slab's prefetch
                hp = sl // 2
                scl_t = wpool.tile([128, 2*QS], f32, tag=f"scl{hp}")
                nc.vector.reciprocal(scl_t[:, :], trp_t[:, :])
                scl[hp] = scl_t

        wb = spool.tile([128, 4096], f16, tag="wb")
        for sl in range(NSLAB):
            if sl % 2 == 0:
                build_wb_left(wb, s2, sl)
            else:
                wbl = wb[:, :].rearrange("p (q j) -> p q j", j=32)
                nc.vector.tensor_scalar_mul(
                    wbl[:, sl*QS:(sl+1)*QS, 0:16],
                    s2[:, SW*sl:SW*(sl+1)].rearrange("p (q j) -> p q j", j=DIM),
                    signp[:, :])
            build_wb_right(wb, s2, sl)

        # ---------------- phase 3: squaring iteration -----------------------
        # E-steps (normalized cast) on odd k; normalizer prefetched on even k.
        for k in range(ksteps):
            last = (k == ksteps - 1)
            # E-steps are STAGGERED by slab-pair (slabs 0-1 normalize on odd
            # steps, slabs 2-3 on even) so every step carries a uniform
            # DVE-cast/Act-cast/prefetch mix; the last step is unnormalized
            # for all slabs (the trace division absorbs the scale)
            s2n = spool.tile([128, 2048], f16, tag="s2")
            wbn = None if last else spool.tile([128, 4096], f16, tag="wb")
            sqn = trpn = None
            if k < ksteps - 2:
                sqn = wpool.tile([128, 1024], f16, tag="sq", name="sqn")
                trpn = ppool_sm.tile([128, 2*QS], f32, tag="sm", name="trpn")
            for sl in range(NSLAB):
                exact = (not last) and k >= 1 and \
                    (k % 2 == (1 if sl < 2 else 0))
                pref = (not last) and k + 1 < ksteps - 1 and \
                    ((k + 1) % 2 == (1 if sl < 2 else 0))
                q0 = sl * QS
                pm = ppool_pm.tile([128, SW], f32, tag="pm")
                for qq in range(QS):
                    q = q0 + qq
                    for s in range(4):
                        nc.tensor.matmul(
                            pm[32*s:32*s+32, 16*qq:16*qq+16],
                            wb[32*s:32*s+32, 32*q:32*q+32],
                            wb[32*s:32*s+32, 32*q:32*q+16],
                            start=True, stop=True,
                            tile_position=(32*s, 32*s))
                dst = s2n[:, SW*sl:SW*(sl+1)].rearrange("p (q j) -> p q j", j=DIM)
                src = pm[:, :].rearrange("p (q j) -> p q j", j=DIM)
                if exact:
                    # cast: s2' = pm * (+-1/s), per-quad scale -> DVE TT
                    nc.vector.tensor_tensor(
                        dst, src,
                        scl[sl // 2][:, QS*(sl % 2):QS*(sl % 2 + 1)]
                            .unsqueeze(-1).broadcast_to([128, QS, DIM]),
                        op=Alu.mult)
                elif last and sl % 2 == 1:
                    # last step: DVE is idle, split the casts with Act
                    nc.vector.tensor_scalar_mul(dst, src, signp[:, :])
                else:
                    # cast: s2' = pm * sign (no normalization this step)
                    nc.scalar.activation(dst, src, Act.Copy, scale=signp[:, :])
                if not last:
                    with tc.high_priority(offset=WB_PRIO):
                        build_wb_left(wbn, s2n, sl)
                        build_wb_right(wbn, s2n, sl)
                    if pref:
                        norm_prefetch(s2n, sl, sqn, trpn)
            s2 = s2n
            if not last:
                wb = wbn

        # ---------------- phase 4: finish (per-slab for pipelining) ---------
        # pos/ea2/trace fold their j-rowsums into PSUM-accumulating matmuls
        # over j-strided state slices: no DVE reduces at all in the tail
        trf = ppool_sm.tile([128, 128], f32, tag="sm")
        pos = ppool_sm.tile([128, 128], f32, tag="sm")
        ea2 = ppool_sm.tile([128, 128], f32, tag="sm")
        def s2j(sl):
            return s2[:, SW*sl:SW*(sl+1)].rearrange("p (q j) -> p j q", j=DIM)
        for w_t, out_t in ((wtrc, trf), (wpos, pos), (wea2, ea2)):
            for sl in range(NSLAB):
                qs = slice(QS*sl, QS*(sl+1))
                for j in range(DIM):
                    wsl = w_t[:, 128*j:128*(j+1)] if w_t is wtrc else w_t[:, :]
                    nc.tensor.matmul(out_t[:, qs], wsl, s2j(sl)[:, j, :],
                                     start=(j == 0), stop=(j == DIM - 1))
        # the rest is column-sliceable; per-slab chains run concurrently on
        # DVE and Pool (PSUM readers stay on DVE; Pool's chain avoids scalar
        # ops -- TensorScalarPtr is illegal there -- via LAMW); the output
        # DMA is split so the first half's latency hides under the second
        invt = wpool.tile([128, 128], f32, tag="invt")
        posn = wpool.tile([128, 128], f32, tag="posn")
        ea2s = wpool.tile([128, 128], f32, tag="ea2s")
        terr = wpool.tile([128, 128], f32, tag="terr")
        t2 = wpool.tile([128, 128], f32, tag="t2")
        np2 = wpool.tile([128, 128], f32, tag="np2")
        r1 = wpool.tile([128, 128], f32, tag="r1")
        r = wpool.tile([128, 128], f32, tag="r")
        outv = wpool.tile([128, 2], f32, tag="outv")
        lamw = wpool.tile([128, 64], f32, tag="lamw")
        nc.gpsimd.memset(lamw[:, :], -LAM)
        for hf in range(2):
            cs = slice(64*hf, 64*hf+64)
            te = nc.vector if hf == 0 else nc.gpsimd
            nc.vector.reciprocal(invt[:, cs], trf[:, cs])
            nc.vector.tensor_tensor(posn[:, cs], pos[:, cs], invt[:, cs],
                                    op=Alu.mult)
            nc.vector.tensor_tensor(ea2s[:, cs], ea2[:, cs], invt[:, cs],
                                    op=Alu.mult)
            # r = terr^2 + ea2s - LAM*posn^2   (LAM baked into WEA2)
            te.tensor_tensor(terr[:, cs], posn[:, cs], xblk[:, cs],
                             op=Alu.subtract)
            te.tensor_tensor(t2[:, cs], terr[:, cs], terr[:, cs], op=Alu.mult)
            if hf == 0:
                nc.vector.scalar_tensor_tensor(np2[:, cs], posn[:, cs], -LAM,
                                               posn[:, cs],
                                               op0=Alu.mult, op1=Alu.mult)
            else:
                nc.gpsimd.tensor_tensor(np2[:, cs], posn[:, cs], posn[:, cs],
                                        op=Alu.mult)
                nc.gpsimd.tensor_tensor(np2[:, cs], np2[:, cs],
                                        lamw[:, :], op=Alu.mult)
            te.tensor_tensor(r1[:, cs], t2[:, cs], ea2s[:, cs], op=Alu.add)
            te.tensor_tensor(r[:, cs], r1[:, cs], np2[:, cs], op=Alu.add)
            nc.vector.tensor_reduce(outv[:, hf:hf+1], r[:, cs],
                                    axis=mybir.AxisListType.X, op=Alu.add)
            nc.sync.dma_start(d_out[:, hf:hf+1], outv[:, hf:hf+1])
    nc.compile()
    return nc


def kernel(A_real, A_imag, X):
    from concourse.bass_utils import run_bass_kernel_spmd

    per_core = _build_host_tensors(
        np.asarray(A_real, np.float32), np.asarray(A_imag, np.float32),
        np.asarray(X, np.float32))

    if "nc" not in _prog_cache:
        _prog_cache["nc"] = build_program()
    nc = _prog_cache["nc"]

    in_maps = [per_core[c] for c in range(NCORES)]
    res = run_bass_kernel_spmd(nc, in_maps, list(range(NCORES)))
    total = 0.0
    for c in range(NCORES):
        total += float(np.asarray(res.results[c]["out"], np.float64).sum())
    loss = total / N
    return np.float32(loss)


# revision 78
# speedup vs baseline: 1.0286x; 1.0009x over previous
"""Trainium2 Bass kernel for nn_EnergyLoss: batched 16x16 complex Hermitian
ground-state projector via shifted matrix-squaring power iteration.

Math summary (all derived from the reference):
  H[n] = 0.5*G - 0.5*sum_d X[n,d]*S_d + (0.5*q_n + EPS)*I,
     G = sum_d A_d A_d^H,  S_d = A_d + A_d^H,  q_n = sum_d X[n,d]^2
  B0 = I - H/(C_SHIFT*||H||_F)   (C_SHIFT tuned so f > (lmax+l1)/2 for every
     sample => |mu(l0)| dominates and the iteration converges to the ground
     state; c<1 tightens the spectral gap and cuts squaring steps)
  B <- B^2, renormalized by tr(B^2)=||B||_F^2 on steps E={1,3,5,7}; the
     normalizer is computed from the state one step EARLIER (on the idle
     N-steps) so the E-step cast never waits on a reduce.
  loss terms from the projector via rowsums (see WPOS/WEA2).
Complex 16x16 matrices are embedded as real 32x32 M(B)=[[Br,-Bi],[Bi,Br]];
per-sample squaring runs as 32x32 PE-array tile matmuls (4 samples per 128
partitions, diagonal tile_positions).  State is fp16, PSUM fp32.

Per-sample 1/(C_SHIFT*fro(H)) is exact on host via the quadratic form
fro^2 = y^T (WH WH^T) y, y=[x;1;q], folded into the XTH columns, so the
device H-matmul directly yields H/f and phase 1 needs no norm pipeline.
Engine split: PE matmuls; DVE psum casts/reduces/shuffles; Act sign-scaled
copies (per-partition scale); Pool state squares + SBUF-only finish ops
(gpsimd cannot touch PSUM or use scalar-broadcast ops on trn2).
"""

import numpy as np

N, D, DIM = 4096, 32, 16
NCORES = 8
NS = N // NCORES          # 512 samples per core
NQ = NS // 4              # 128 quads (4 samples stacked per 128 partitions)
EPS = 1e-5
LAM = 0.1
C_SHIFT = 0.28
KSTEPS = 8
NSLAB = 4                 # quad slabs for pipelining
QS = NQ // NSLAB          # 32 quads per slab

_prog_cache = {}

# packed constant-input byte offsets (per partition), ordered by DMA urgency:
# DMA1 = signs+XTH+WH (H matmuls + first casts), DMA2 = DIAGP (phase-1 STT),
# DMA3 = the rest (first needed by the k=0 normalizer prefetch)
OFF_SIGNP = 0         # f32 [128,1]    4B
OFF_SIGNPM = 4        # f32 [128,1]    4B
OFF_XTH = 16          # f16 [34,512]   1024B  (columns pre-scaled by 1/f_n)
OFF_WH = 1040         # f16 [34,512]   1024B
OFF_DIAGP = 2064      # f16 [128,2048] 4096B
OFF_MASKBS = 6160     # f16 [128,128]  256B   signed block mask (+-1 rows)
OFF_SIGNW = 6416      # f16 [128,512]  1024B  per-partition sign, 512 wide
OFF_XBLK = 7440       # f32 [128,128]  512B
OFF_WPOS = 7952       # f16 [128,128]  256B
OFF_WEA2 = 8208       # f16 [128,128]  256B   (pre-scaled by LAMBDA_REG)
OFF_WTRC = 8464       # f16 [128,16*128] 4096B  per-j trace-pick block masks
OFF_MASKBSF = 12560   # f32 [128,128]  512B   signed mask, f32 for DVE path
OFF_WPOSF = 13072     # f32 [128,128]  512B
OFF_WEA2F = 13584     # f32 [128,128]  512B
CIN_BYTES = 14096
DMA1_HI = OFF_DIAGP
DMA2_HI = OFF_MASKBS
HSWAP = list(range(16, 32)) + list(range(0, 16))


def _build_host_tensors(A_real, A_imag, X):
    """All small A-derived tensors + per-core X-derived layouts (numpy fp32)."""
    A = (A_real + 1j * A_imag).astype(np.complex64)
    Sc = A + np.conj(np.transpose(A, (0, 2, 1)))        # [D,16,16] Hermitian
    Sr, Si = Sc.real.astype(np.float64), Sc.imag.astype(np.float64)
    G = np.einsum('dij,dkj->ik', A, A.conj())
    Gr, Gi = G.real.astype(np.float64), G.imag.astype(np.float64)
    cA = A.sum(axis=1)                                   # [D,16] colsum over i
    cA2 = (A @ A).sum(axis=1)

    # H-build weights: WH[k, 32j+m], contraction k: 0..31 = d, 32 = const, 33 = q
    WH = np.zeros((34, 512), np.float64)
    for j in range(DIM):
        c = 32 * j
        WH[:D, c:c+16] = -0.5 * Sr[:, :, j]              # m<16 -> Hr[m,j]
        WH[:D, c+16:c+32] = -0.5 * Si[:, :, j]           # m>=16 -> Hi[m-16,j]
        WH[32, c:c+16] = 0.5 * Gr[:, j]
        WH[32, c+j] += EPS
        WH[32, c+16:c+32] = 0.5 * Gi[:, j]
        WH[33, c+j] = 0.5
    MQF = WH @ WH.T                                      # fro^2 quadratic form
    # diag delta pattern on the state layout (top halves only)
    DIAGP = np.zeros((128, 16 * NQ), np.float32)
    for s in range(4):
        for i in range(DIM):
            DIAGP[32*s + i, i::16] = 1.0
    # block masks for cross-partition per-sample sums; signed variant bakes
    # the state's [+ / -] partition sign into the broadcast normalizer.
    # WTRC[p, 128j+p'] picks the j-th diagonal row of each block (trace).
    MASKBS = np.zeros((128, 128), np.float32)
    WTRC = np.zeros((128, 16 * 128), np.float32)
    SIGNP = np.ones((128, 1), np.float32)
    for s in range(4):
        SIGNP[32*s+16:32*s+32, 0] = -1.0
    for b in range(4):
        MASKBS[32*b:32*b+32, 32*b:32*b+32] = SIGNP[32*b:32*b+32, 0][None, :]
        for j in range(DIM):
            WTRC[32*b+j, 128*j+32*b:128*j+32*b+32] = 1.0
    # finish functionals: rhs is RS from s2 = [Pr; -Pi] rowsums
    #   pos_raw[32s+d, q] = sum_i cAr[d,i]*rr[i] - cAi[d,i]*ri[i]
    #   RS bottom rows hold -ri  =>  bottom weight = +cAi
    WPOS = np.zeros((128, 128), np.float32)
    WEA2 = np.zeros((128, 128), np.float32)
    for s in range(4):
        b = 32 * s
        WPOS[b:b+16, b:b+32] = cA.real.T                 # [i, d]
        WPOS[b+16:b+32, b:b+32] = cA.imag.T
        WEA2[b:b+16, b:b+32] = LAM * cA2.real.T
        WEA2[b+16:b+32, b:b+32] = LAM * cA2.imag.T

    def put(buf, rows, off, arr):
        b = np.ascontiguousarray(arr).view(np.uint8).reshape(arr.shape[0], -1)
        buf[:rows, off:off+b.shape[1]] = b

    per_core = []
    for c in range(NCORES):
        Xc = np.asarray(X[c*NS:(c+1)*NS], np.float64)    # [512, 32]
        q = (Xc ** 2).sum(1)
        Y = np.concatenate([Xc, np.ones((NS, 1)), q[:, None]], axis=1)  # [512,34]
        fro = np.sqrt(np.einsum('nk,kl,nl->n', Y, MQF, Y))
        invf = 1.0 / (C_SHIFT * fro)                     # [512]
        YS = (Y * invf[:, None]).astype(np.float32)      # scaled y columns
        XTH = np.zeros((34, 512), np.float32)
        XBLK = np.zeros((128, 128), np.float32)
        for s in range(4):
            idx = np.arange(NQ) * 4 + s                  # n_core(q,s)
            XTH[:, 128*s:128*s+128] = YS[idx].T
            XBLK[32*s:32*s+32, :] = Xc[idx].T
        buf = np.zeros((128, CIN_BYTES), np.uint8)
        put(buf, 128, OFF_XBLK, XBLK)
        put(buf, 128, OFF_MASKBS, MASKBS.astype(np.float16))
        put(buf, 128, OFF_SIGNW, np.repeat(SIGNP, 512, 1).astype(np.float16))
        put(buf, 128, OFF_SIGNP, SIGNP)
        put(buf, 128, OFF_SIGNPM, -SIGNP)
        put(buf, 34, OFF_XTH, XTH.astype(np.float16))
        put(buf, 34, OFF_WH, WH.astype(np.float16))
        put(buf, 128, OFF_WPOS, WPOS.astype(np.float16))
        put(buf, 128, OFF_WEA2, WEA2.astype(np.float16))
        put(buf, 128, OFF_WTRC, WTRC.astype(np.float16))
        put(buf, 128, OFF_MASKBSF, MASKBS)
        put(buf, 128, OFF_WPOSF, WPOS)
        put(buf, 128, OFF_WEA2F, WEA2)
        put(buf, 128, OFF_DIAGP, DIAGP.astype(np.float16))
        per_core.append({"cin": buf})
    return per_core


def build_program(ksteps=KSTEPS, warmup=20, PREF_DVE_SLABS=1, WB_PRIO=0):
    import concourse.bass as bass
    import concourse.bacc as bacc
    import concourse.mybir as mybir
    import concourse.tile as tile
    from contextlib import ExitStack

    f16, f32 = mybir.dt.float16, mybir.dt.float32
    u8, u32 = mybir.dt.uint8, mybir.dt.uint32
    Alu = mybir.AluOpType
    Act = mybir.ActivationFunctionType

    nc = bacc.Bacc()
    d_cin = nc.dram_tensor("cin", [128, CIN_BYTES], u8, kind="ExternalInput")
    d_out = nc.dram_tensor("out", [128, 2], f32, kind="ExternalOutput")

    SW = 16 * QS              # state cols per slab (512)

    with tile.TileContext(nc) as tc, ExitStack() as ctx:
        cpool = ctx.enter_context(tc.tile_pool(name="consts", bufs=1))
        spool = ctx.enter_context(tc.tile_pool(name="state", bufs=3))
        wpool = ctx.enter_context(tc.tile_pool(name="work", bufs=2))
        ppool_pm = ctx.enter_context(tc.tile_pool(name="psum_pm", bufs=6, space="PSUM"))
        ppool_sm = ctx.enter_context(tc.tile_pool(name="psum_sm", bufs=2, space="PSUM"))

        # ---------------- input DMAs (critical slice first) -----------------
        cst = cpool.tile([128, CIN_BYTES], u8, tag="cin")
        nc.sync.dma_start(cst[:, 0:DMA1_HI], d_cin[:, 0:DMA1_HI])
        nc.sync.dma_start(cst[:, DMA1_HI:DMA1_HI+1024],
                          d_cin[:, DMA1_HI:DMA1_HI+1024])
        nc.sync.dma_start(cst[:, DMA1_HI+1024:DMA2_HI],
                          d_cin[:, DMA1_HI+1024:DMA2_HI])
        nc.sync.dma_start(cst[:, DMA2_HI:], d_cin[:, DMA2_HI:])
        xblk = cst[:, OFF_XBLK:OFF_XBLK+512].bitcast(f32)
        maskbs = cst[:, OFF_MASKBS:OFF_MASKBS+256].bitcast(f16)
        signw = cst[:, OFF_SIGNW:OFF_SIGNW+1024].bitcast(f16)
        signp = cst[:, OFF_SIGNP:OFF_SIGNP+4].bitcast(f32)
        signpm = cst[:, OFF_SIGNPM:OFF_SIGNPM+4].bitcast(f32)
        xth = cst[:, OFF_XTH:OFF_XTH+1024].bitcast(f16)[0:34, :]
        wh = cst[:, OFF_WH:OFF_WH+1024].bitcast(f16)[0:34, :]
        wpos = cst[:, OFF_WPOS:OFF_WPOS+256].bitcast(f16)
        wea2 = cst[:, OFF_WEA2:OFF_WEA2+256].bitcast(f16)
        wtrc = cst[:, OFF_WTRC:OFF_WTRC+4096].bitcast(f16)
        maskbsf = cst[:, OFF_MASKBSF:OFF_MASKBSF+512].bitcast(f32)
        wposf = cst[:, OFF_WPOSF:OFF_WPOSF+512].bitcast(f32)
        wea2f = cst[:, OFF_WEA2F:OFF_WEA2F+512].bitcast(f32)
        diagp = cst[:, OFF_DIAGP:OFF_DIAGP+4096].bitcast(f16)

        # ---------------- PE p-state warmup during the DMA ------------------
        if warmup:
            wz = wpool.tile([128, 128], f16, tag="warm")
            nc.gpsimd.memset(wz[:, :], 0)
            wz2 = wpool.tile([128, 128], f16, tag="warm2")
            nc.scalar.activation(wz2[:, :], wz[:, :], Act.Copy)
            pwarm = ppool_pm.tile([128, 512], f32, tag="pm")
            for i in range(warmup):
                nc.tensor.matmul(pwarm[:, 128*(i % 4):128*(i % 4)+128],
                                 wz[:, :], wz[:, :], start=True, stop=True)

        # ---------------- phase 1: H/f build, B0 = I - H/f ------------------
        # Quad-sliced by slab so slab 0's state (and its wb + step-0 matmuls)
        # flows while slabs 1-3 are still streaming H.
        s2 = spool.tile([128, 2048], f16, tag="s2")
        p1t = wpool.tile([128, 2048], f16, tag="p1t")
        for sl in range(NSLAB):
            ph = ppool_pm.tile([128, SW], f32, tag="pm")
            for j in range(DIM):
                for s in range(4):
                    nc.tensor.matmul(
                        ph[32*s:32*s+32, QS*j:QS*(j+1)],
                        wh[:, 32*j:32*j+32],
                        xth[:, 128*s+QS*sl:128*s+QS*(sl+1)],
                        start=True, stop=True,
                        tile_position=(0, 32*s),
                    )
            # s2 slab = diagp + pm*signpm: Act does the psum read + sign,
            # DVE adds the diagonal (pipelines across slabs)
            nc.scalar.activation(
                p1t[:, SW*sl:SW*(sl+1)].rearrange("p (q j) -> p q j", j=DIM),
                ph[:, :].rearrange("p (j q) -> p q j", j=DIM),
                Act.Copy, scale=signpm[:, :])
            nc.vector.tensor_tensor(s2[:, SW*sl:SW*(sl+1)],
                                    p1t[:, SW*sl:SW*(sl+1)],
                                    diagp[:, SW*sl:SW*(sl+1)], op=Alu.add)

        def build_wb_left(wb_t, s2_t, sl):
            """wb[:, 32q+0:16] = s2*signp (-> [Br;Bi]).  Alternate slabs go to
            Act (per-partition scale) and Pool (real-tensor multiply by the
            materialized sign plane -- Pool can't use scalar-broadcast ops)."""
            wbl = wb_t[:, :].rearrange("p (q j) -> p q j", j=32)
            src = s2_t[:, SW*sl:SW*(sl+1)].rearrange("p (q j) -> p q j", j=DIM)
            dst = wbl[:, sl*QS:(sl+1)*QS, 0:16]
            if sl % 2 == 0:
                nc.scalar.activation(dst, src, Act.Copy, scale=signp[:, :])
            else:
                nc.gpsimd.tensor_tensor(
                    dst, src,
                    signw[:, 0:SW].rearrange("p (q j) -> p q j", j=DIM),
                    op=Alu.mult)

        def build_wb_right(wb_t, s2_t, sl):
            """wb[:, 32q+16:32] = partition-half-swapped s2 (-> [-Bi;Br])."""
            wbw = wb_t[:, :].bitcast(u32).rearrange("p (q w) -> p q w", w=16)
            s2w = s2_t[:, :].bitcast(u32)
            nc.vector.stream_shuffle(
                wbw[:, sl*QS:(sl+1)*QS, 8:16],
                s2w[:, 8*sl*QS:8*(sl+1)*QS].rearrange("p (q w) -> p q w", w=8),
                mask=HSWAP)

        scl = [None, None]    # per slab-pair: [128, 64] tiles of +-1/s

        def norm_prefetch(s2_t, sl, sq_t, trp_t):
            """Normalizer for the NEXT step's cast: fro^2 of the state slab.
            Square on Pool (SBUF only); the (partition-block x j) double sum
            runs as 16 PSUM-accumulating matmuls over j-strided rhs slices
            with the sign-baked mask weights -- zero DVE cost; one batched
            recip per slab-pair then yields +-1/s."""
            c0, c1 = SW*sl, SW*(sl+1)
            # the normalizer only controls fp16 range, so sample every
            # FOURTH j-column: quarter the square/block-sum volume (stride 8
            # overflows fp16 on the even-E schedule; 4 holds with margin)
            JH2 = DIM // 4
            sc0 = JH2 * QS * sl
            seven = s2_t[:, c0:c1].rearrange(
                "p (q j four) -> p q j four", j=JH2, four=4)[:, :, :, 0]
            sqc = sq_t[:, sc0:sc0+JH2*QS].rearrange("p (q j) -> p q j", j=JH2)
            nc.vector.tensor_tensor(sqc, seven, seven, op=Alu.mult)
            slot = sl % 2
            if sl >= PREF_DVE_SLABS:
                sqj = sq_t[:, sc0:sc0+JH2*QS].rearrange(
                    "p (q j) -> p j q", j=JH2)
                for j in range(JH2):
                    nc.tensor.matmul(trp_t[:, QS*slot:QS*(slot+1)],
                                     maskbs[:, :], sqj[:, j, :],
                                     start=(j == 0), stop=(j == JH2 - 1))
            else:
                pr = wpool.tile([128, QS], f32, tag=f"pr{sl}")
                nc.vector.tensor_reduce(
                    pr[:, :],
                    sq_t[:, sc0:sc0+JH2*QS].rearrange("p (q j) -> p q j",
                                                      j=JH2),
                    axis=mybir.AxisListType.X, op=Alu.add)
                nc.tensor.matmul(trp_t[:, QS*slot:QS*(slot+1)], maskbsf[:, :],
                                 pr[:, :], start=True, stop=True)
            if slot == 1:
                # recip per slab-pair: ready mid-step, so next step's E-casts
                # for these slabs never wait on the tail slab's prefetch
                hp = sl // 2
                scl_t = wpool.tile([128, 2*QS], f32, tag=f"scl{hp}")
                nc.vector.reciprocal(scl_t[:, :], trp_t[:, :])
                scl[hp] = scl_t

        wb = spool.tile([128, 4096], f16, tag="wb")
        for sl in range(NSLAB):
            if sl % 2 == 0:
                build_wb_left(wb, s2, sl)
            else:
                wbl = wb[:, :].rearrange("p (q j) -> p q j", j=32)
                nc.vector.tensor_scalar_mul(
                    wbl[:, sl*QS:(sl+1)*QS, 0:16],
                    s2[:, SW*sl:SW*(sl+1)].rearrange("p (q j) -> p q j", j=DIM),
                    signp[:, :])
            build_wb_right(wb, s2, sl)

        # ---------------- phase 3: squaring iteration -----------------------
        # E-steps (normalized cast) on odd k; normalizer prefetched on even k.
        for k in range(ksteps):
            last = (k == ksteps - 1)
            # E-steps are STAGGERED by slab-pair (slabs 0-1 normalize on odd
            # steps, slabs 2-3 on even) so every step carries a uniform
            # DVE-cast/Act-cast/prefetch mix; the last step is unnormalized
            # for all slabs (the trace division absorbs the scale)
            s2n = spool.tile([128, 2048], f16, tag="s2")
            wbn = None if last else spool.tile([128, 4096], f16, tag="wb")
            sqn = trpn = None
            if k < ksteps - 2:
                sqn = wpool.tile([128, 1024], f16, tag="sq", name="sqn")
                trpn = ppool_sm.tile([128, 2*QS], f32, tag="sm", name="trpn")
            for sl in range(NSLAB):
                exact = (not last) and k >= 1 and \
                    (k % 2 == (1 if sl < 2 else 0))
                pref = (not last) and k + 1 < ksteps - 1 and \
                    ((k + 1) % 2 == (1 if sl < 2 else 0))
                q0 = sl * QS
                pm = ppool_pm.tile([128, SW], f32, tag="pm")
                for qq in range(QS):
                    q = q0 + qq
                    for s in range(4):
                        nc.tensor.matmul(
                            pm[32*s:32*s+32, 16*qq:16*qq+16],
                            wb[32*s:32*s+32, 32*q:32*q+32],
                            wb[32*s:32*s+32, 32*q:32*q+16],
                            start=True, stop=True,
                            tile_position=(32*s, 32*s))
                dst = s2n[:, SW*sl:SW*(sl+1)].rearrange("p (q j) -> p q j", j=DIM)
                src = pm[:, :].rearrange("p (q j) -> p q j", j=DIM)
                if exact:
                    # cast: s2' = pm * (+-1/s), per-quad scale -> DVE TT
                    nc.vector.tensor_tensor(
                        dst, src,
                        scl[sl // 2][:, QS*(sl % 2):QS*(sl % 2 + 1)]
                            .unsqueeze(-1).broadcast_to([128, QS, DIM]),
                        op=Alu.mult)
                elif last and sl % 2 == 1:
                    # last step: DVE is idle, split the casts with Act
                    nc.vector.tensor_scalar_mul(dst, src, signp[:, :])
                else:
                    # cast: s2' = pm * sign (no normalization this step)
                    nc.scalar.activation(dst, src, Act.Copy, scale=signp[:, :])
                if not last:
                    with tc.high_priority(offset=WB_PRIO):
                        build_wb_right(wbn, s2n, sl)
                        build_wb_left(wbn, s2n, sl)
                    if pref:
                        norm_prefetch(s2n, sl, sqn, trpn)
            s2 = s2n
            if not last:
                wb = wbn

        # ---------------- phase 4: finish (per-slab for pipelining) ---------
        # pos/ea2/trace fold their j-rowsums into PSUM-accumulating matmuls
        # over j-strided state slices: no DVE reduces at all in the tail
        trf = ppool_sm.tile([128, 128], f32, tag="sm")
        pos = ppool_sm.tile([128, 128], f32, tag="sm")
        ea2 = ppool_sm.tile([128, 128], f32, tag="sm")
        def s2j(sl):
            return s2[:, SW*sl:SW*(sl+1)].rearrange("p (q j) -> p j q", j=DIM)
        for w_t, out_t in ((wtrc, trf), (wpos, pos), (wea2, ea2)):
            for sl in range(NSLAB):
                qs = slice(QS*sl, QS*(sl+1))
                for j in range(DIM):
                    wsl = w_t[:, 128*j:128*(j+1)] if w_t is wtrc else w_t[:, :]
                    nc.tensor.matmul(out_t[:, qs], wsl, s2j(sl)[:, j, :],
                                     start=(j == 0), stop=(j == DIM - 1))
        # the rest is column-sliceable; per-slab chains run concurrently on
        # DVE and Pool (PSUM readers stay on DVE; Pool's chain avoids scalar
        # ops -- TensorScalarPtr is illegal there -- via LAMW); the output
        # DMA is split so the first half's latency hides under the second
        invt = wpool.tile([128, 128], f32, tag="invt")
        posn = wpool.tile([128, 128], f32, tag="posn")
        ea2s = wpool.tile([128, 128], f32, tag="ea2s")
        terr = wpool.tile([128, 128], f32, tag="terr")
        t2 = wpool.tile([128, 128], f32, tag="t2")
        np2 = wpool.tile([128, 128], f32, tag="np2")
        r1 = wpool.tile([128, 128], f32, tag="r1")
        r = wpool.tile([128, 128], f32, tag="r")
        outv = wpool.tile([128, 2], f32, tag="outv")
        lamw = wpool.tile([128, 64], f32, tag="lamw")
        nc.gpsimd.memset(lamw[:, :], -LAM)
        for hf in range(2):
            cs = slice(64*hf, 64*hf+64)
            te = nc.vector if hf == 0 else nc.gpsimd
            nc.vector.reciprocal(invt[:, cs], trf[:, cs])
            nc.vector.tensor_tensor(posn[:, cs], pos[:, cs], invt[:, cs],
                                    op=Alu.mult)
            nc.vector.tensor_tensor(ea2s[:, cs], ea2[:, cs], invt[:, cs],
                                    op=Alu.mult)
            # r = terr^2 + ea2s - LAM*posn^2   (LAM baked into WEA2)
            te.tensor_tensor(terr[:, cs], posn[:, cs], xblk[:, cs],
                             op=Alu.subtract)
            te.tensor_tensor(t2[:, cs], terr[:, cs], terr[:, cs], op=Alu.mult)
            if hf == 0:
                nc.vector.scalar_tensor_tensor(np2[:, cs], posn[:, cs], -LAM,
                                               posn[:, cs],
                                               op0=Alu.mult, op1=Alu.mult)
            else:
                nc.gpsimd.tensor_tensor(np2[:, cs], posn[:, cs], posn[:, cs],
                                        op=Alu.mult)
                nc.gpsimd.tensor_tensor(np2[:, cs], np2[:, cs],
                                        lamw[:, :], op=Alu.mult)
            te.tensor_tensor(r1[:, cs], t2[:, cs], ea2s[:, cs], op=Alu.add)
            te.tensor_tensor(r[:, cs], r1[:, cs], np2[:, cs], op=Alu.add)
            nc.vector.tensor_reduce(outv[:, hf:hf+1], r[:, cs],
                                    axis=mybir.AxisListType.X, op=Alu.add)
            nc.sync.dma_start(d_out[:, hf:hf+1], outv[:, hf:hf+1])
    nc.compile()
    return nc


def kernel(A_real, A_imag, X):
    from concourse.bass_utils import run_bass_kernel_spmd

    per_core = _build_host_tensors(
        np.asarray(A_real, np.float32), np.asarray(A_imag, np.float32),
        np.asarray(X, np.float32))

    if "nc" not in _prog_cache:
        _prog_cache["nc"] = build_program()
    nc = _prog_cache["nc"]

    in_maps = [per_core[c] for c in range(NCORES)]
    res = run_bass_kernel_spmd(nc, in_maps, list(range(NCORES)))
    total = 0.0
    for c in range(NCORES):
        total += float(np.asarray(res.results[c]["out"], np.float64).sum())
    loss = total / N
    return np.float32(loss)


# revision 79
# speedup vs baseline: 1.0297x; 1.0011x over previous
"""Trainium2 Bass kernel for nn_EnergyLoss: batched 16x16 complex Hermitian
ground-state projector via shifted matrix-squaring power iteration.

Math summary (all derived from the reference):
  H[n] = 0.5*G - 0.5*sum_d X[n,d]*S_d + (0.5*q_n + EPS)*I,
     G = sum_d A_d A_d^H,  S_d = A_d + A_d^H,  q_n = sum_d X[n,d]^2
  B0 = I - H/(C_SHIFT*||H||_F)   (C_SHIFT tuned so f > (lmax+l1)/2 for every
     sample => |mu(l0)| dominates and the iteration converges to the ground
     state; c<1 tightens the spectral gap and cuts squaring steps)
  B <- B^2, renormalized by tr(B^2)=||B||_F^2 on steps E={1,3,5,7}; the
     normalizer is computed from the state one step EARLIER (on the idle
     N-steps) so the E-step cast never waits on a reduce.
  loss terms from the projector via rowsums (see WPOS/WEA2).
Complex 16x16 matrices are embedded as real 32x32 M(B)=[[Br,-Bi],[Bi,Br]];
per-sample squaring runs as 32x32 PE-array tile matmuls (4 samples per 128
partitions, diagonal tile_positions).  State is fp16, PSUM fp32.

Per-sample 1/(C_SHIFT*fro(H)) is exact on host via the quadratic form
fro^2 = y^T (WH WH^T) y, y=[x;1;q], folded into the XTH columns, so the
device H-matmul directly yields H/f and phase 1 needs no norm pipeline.
Engine split: PE matmuls; DVE psum casts/reduces/shuffles; Act sign-scaled
copies (per-partition scale); Pool state squares + SBUF-only finish ops
(gpsimd cannot touch PSUM or use scalar-broadcast ops on trn2).
"""

import numpy as np

N, D, DIM = 4096, 32, 16
NCORES = 8
NS = N // NCORES          # 512 samples per core
NQ = NS // 4              # 128 quads (4 samples stacked per 128 partitions)
EPS = 1e-5
LAM = 0.1
C_SHIFT = 0.28
KSTEPS = 8
NSLAB = 4                 # quad slabs for pipelining
QS = NQ // NSLAB          # 32 quads per slab

_prog_cache = {}

# packed constant-input byte offsets (per partition), ordered by DMA urgency:
# DMA1 = signs+XTH+WH (H matmuls + first casts), DMA2 = DIAGP (phase-1 STT),
# DMA3 = the rest (first needed by the k=0 normalizer prefetch)
OFF_SIGNP = 0         # f32 [128,1]    4B
OFF_SIGNPM = 4        # f32 [128,1]    4B
OFF_XTH = 16          # f16 [34,512]   1024B  (columns pre-scaled by 1/f_n)
OFF_WH = 1040         # f16 [34,512]   1024B
OFF_DIAGP = 2064      # f16 [128,2048] 4096B
OFF_MASKBS = 6160     # f16 [128,128]  256B   signed block mask (+-1 rows)
OFF_SIGNW = 6416      # f16 [128,512]  1024B  per-partition sign, 512 wide
OFF_XBLK = 7440       # f32 [128,128]  512B
OFF_WPOS = 7952       # f16 [128,128]  256B
OFF_WEA2 = 8208       # f16 [128,128]  256B   (pre-scaled by LAMBDA_REG)
OFF_WTRC = 8464       # f16 [128,16*128] 4096B  per-j trace-pick block masks
OFF_MASKBSF = 12560   # f32 [128,128]  512B   signed mask, f32 for DVE path
OFF_WPOSF = 13072     # f32 [128,128]  512B
OFF_WEA2F = 13584     # f32 [128,128]  512B
CIN_BYTES = 14096
DMA1_HI = OFF_DIAGP
DMA2_HI = OFF_MASKBS
HSWAP = list(range(16, 32)) + list(range(0, 16))


def _build_host_tensors(A_real, A_imag, X):
    """All small A-derived tensors + per-core X-derived layouts (numpy fp32)."""
    A = (A_real + 1j * A_imag).astype(np.complex64)
    Sc = A + np.conj(np.transpose(A, (0, 2, 1)))        # [D,16,16] Hermitian
    Sr, Si = Sc.real.astype(np.float64), Sc.imag.astype(np.float64)
    G = np.einsum('dij,dkj->ik', A, A.conj())
    Gr, Gi = G.real.astype(np.float64), G.imag.astype(np.float64)
    cA = A.sum(axis=1)                                   # [D,16] colsum over i
    cA2 = (A @ A).sum(axis=1)

    # H-build weights: WH[k, 32j+m], contraction k: 0..31 = d, 32 = const, 33 = q
    WH = np.zeros((34, 512), np.float64)
    for j in range(DIM):
        c = 32 * j
        WH[:D, c:c+16] = -0.5 * Sr[:, :, j]              # m<16 -> Hr[m,j]
        WH[:D, c+16:c+32] = -0.5 * Si[:, :, j]           # m>=16 -> Hi[m-16,j]
        WH[32, c:c+16] = 0.5 * Gr[:, j]
        WH[32, c+j] += EPS
        WH[32, c+16:c+32] = 0.5 * Gi[:, j]
        WH[33, c+j] = 0.5
    MQF = WH @ WH.T                                      # fro^2 quadratic form
    # diag delta pattern on the state layout (top halves only)
    DIAGP = np.zeros((128, 16 * NQ), np.float32)
    for s in range(4):
        for i in range(DIM):
            DIAGP[32*s + i, i::16] = 1.0
    # block masks for cross-partition per-sample sums; signed variant bakes
    # the state's [+ / -] partition sign into the broadcast normalizer.
    # WTRC[p, 128j+p'] picks the j-th diagonal row of each block (trace).
    MASKBS = np.zeros((128, 128), np.float32)
    WTRC = np.zeros((128, 16 * 128), np.float32)
    SIGNP = np.ones((128, 1), np.float32)
    for s in range(4):
        SIGNP[32*s+16:32*s+32, 0] = -1.0
    for b in range(4):
        MASKBS[32*b:32*b+32, 32*b:32*b+32] = SIGNP[32*b:32*b+32, 0][None, :]
        for j in range(DIM):
            WTRC[32*b+j, 128*j+32*b:128*j+32*b+32] = 1.0
    # finish functionals: rhs is RS from s2 = [Pr; -Pi] rowsums
    #   pos_raw[32s+d, q] = sum_i cAr[d,i]*rr[i] - cAi[d,i]*ri[i]
    #   RS bottom rows hold -ri  =>  bottom weight = +cAi
    WPOS = np.zeros((128, 128), np.float32)
    WEA2 = np.zeros((128, 128), np.float32)
    for s in range(4):
        b = 32 * s
        WPOS[b:b+16, b:b+32] = cA.real.T                 # [i, d]
        WPOS[b+16:b+32, b:b+32] = cA.imag.T
        WEA2[b:b+16, b:b+32] = LAM * cA2.real.T
        WEA2[b+16:b+32, b:b+32] = LAM * cA2.imag.T

    def put(buf, rows, off, arr):
        b = np.ascontiguousarray(arr).view(np.uint8).reshape(arr.shape[0], -1)
        buf[:rows, off:off+b.shape[1]] = b

    per_core = []
    for c in range(NCORES):
        Xc = np.asarray(X[c*NS:(c+1)*NS], np.float64)    # [512, 32]
        q = (Xc ** 2).sum(1)
        Y = np.concatenate([Xc, np.ones((NS, 1)), q[:, None]], axis=1)  # [512,34]
        fro = np.sqrt(np.einsum('nk,kl,nl->n', Y, MQF, Y))
        invf = 1.0 / (C_SHIFT * fro)                     # [512]
        YS = (Y * invf[:, None]).astype(np.float32)      # scaled y columns
        XTH = np.zeros((34, 512), np.float32)
        XBLK = np.zeros((128, 128), np.float32)
        for s in range(4):
            idx = np.arange(NQ) * 4 + s                  # n_core(q,s)
            XTH[:, 128*s:128*s+128] = YS[idx].T
            XBLK[32*s:32*s+32, :] = Xc[idx].T
        buf = np.zeros((128, CIN_BYTES), np.uint8)
        put(buf, 128, OFF_XBLK, XBLK)
        put(buf, 128, OFF_MASKBS, MASKBS.astype(np.float16))
        put(buf, 128, OFF_SIGNW, np.repeat(SIGNP, 512, 1).astype(np.float16))
        put(buf, 128, OFF_SIGNP, SIGNP)
        put(buf, 128, OFF_SIGNPM, -SIGNP)
        put(buf, 34, OFF_XTH, XTH.astype(np.float16))
        put(buf, 34, OFF_WH, WH.astype(np.float16))
        put(buf, 128, OFF_WPOS, WPOS.astype(np.float16))
        put(buf, 128, OFF_WEA2, WEA2.astype(np.float16))
        put(buf, 128, OFF_WTRC, WTRC.astype(np.float16))
        put(buf, 128, OFF_MASKBSF, MASKBS)
        put(buf, 128, OFF_WPOSF, WPOS)
        put(buf, 128, OFF_WEA2F, WEA2)
        put(buf, 128, OFF_DIAGP, DIAGP.astype(np.float16))
        per_core.append({"cin": buf})
    return per_core


def build_program(ksteps=KSTEPS, warmup=20, PREF_DVE_SLABS=1, WB_PRIO=0):
    import concourse.bass as bass
    import concourse.bacc as bacc
    import concourse.mybir as mybir
    import concourse.tile as tile
    from contextlib import ExitStack

    f16, f32 = mybir.dt.float16, mybir.dt.float32
    u8, u32 = mybir.dt.uint8, mybir.dt.uint32
    Alu = mybir.AluOpType
    Act = mybir.ActivationFunctionType

    nc = bacc.Bacc()
    d_cin = nc.dram_tensor("cin", [128, CIN_BYTES], u8, kind="ExternalInput")
    d_out = nc.dram_tensor("out", [128, 2], f32, kind="ExternalOutput")

    SW = 16 * QS              # state cols per slab (512)

    with tile.TileContext(nc) as tc, ExitStack() as ctx:
        cpool = ctx.enter_context(tc.tile_pool(name="consts", bufs=1))
        spool = ctx.enter_context(tc.tile_pool(name="state", bufs=4))
        wpool = ctx.enter_context(tc.tile_pool(name="work", bufs=2))
        ppool_pm = ctx.enter_context(tc.tile_pool(name="psum_pm", bufs=6, space="PSUM"))
        ppool_sm = ctx.enter_context(tc.tile_pool(name="psum_sm", bufs=2, space="PSUM"))

        # ---------------- input DMAs (critical slice first) -----------------
        cst = cpool.tile([128, CIN_BYTES], u8, tag="cin")
        nc.sync.dma_start(cst[:, 0:DMA1_HI], d_cin[:, 0:DMA1_HI])
        nc.sync.dma_start(cst[:, DMA1_HI:DMA1_HI+1024],
                          d_cin[:, DMA1_HI:DMA1_HI+1024])
        nc.sync.dma_start(cst[:, DMA1_HI+1024:DMA2_HI],
                          d_cin[:, DMA1_HI+1024:DMA2_HI])
        nc.sync.dma_start(cst[:, DMA2_HI:], d_cin[:, DMA2_HI:])
        xblk = cst[:, OFF_XBLK:OFF_XBLK+512].bitcast(f32)
        maskbs = cst[:, OFF_MASKBS:OFF_MASKBS+256].bitcast(f16)
        signw = cst[:, OFF_SIGNW:OFF_SIGNW+1024].bitcast(f16)
        signp = cst[:, OFF_SIGNP:OFF_SIGNP+4].bitcast(f32)
        signpm = cst[:, OFF_SIGNPM:OFF_SIGNPM+4].bitcast(f32)
        xth = cst[:, OFF_XTH:OFF_XTH+1024].bitcast(f16)[0:34, :]
        wh = cst[:, OFF_WH:OFF_WH+1024].bitcast(f16)[0:34, :]
        wpos = cst[:, OFF_WPOS:OFF_WPOS+256].bitcast(f16)
        wea2 = cst[:, OFF_WEA2:OFF_WEA2+256].bitcast(f16)
        wtrc = cst[:, OFF_WTRC:OFF_WTRC+4096].bitcast(f16)
        maskbsf = cst[:, OFF_MASKBSF:OFF_MASKBSF+512].bitcast(f32)
        wposf = cst[:, OFF_WPOSF:OFF_WPOSF+512].bitcast(f32)
        wea2f = cst[:, OFF_WEA2F:OFF_WEA2F+512].bitcast(f32)
        diagp = cst[:, OFF_DIAGP:OFF_DIAGP+4096].bitcast(f16)

        # ---------------- PE p-state warmup during the DMA ------------------
        if warmup:
            wz = wpool.tile([128, 128], f16, tag="warm")
            nc.gpsimd.memset(wz[:, :], 0)
            wz2 = wpool.tile([128, 128], f16, tag="warm2")
            nc.scalar.activation(wz2[:, :], wz[:, :], Act.Copy)
            pwarm = ppool_pm.tile([128, 512], f32, tag="pm")
            for i in range(warmup):
                nc.tensor.matmul(pwarm[:, 128*(i % 4):128*(i % 4)+128],
                                 wz[:, :], wz[:, :], start=True, stop=True)

        # ---------------- phase 1: H/f build, B0 = I - H/f ------------------
        # Quad-sliced by slab so slab 0's state (and its wb + step-0 matmuls)
        # flows while slabs 1-3 are still streaming H.
        s2 = spool.tile([128, 2048], f16, tag="s2")
        p1t = wpool.tile([128, 2048], f16, tag="p1t")
        for sl in range(NSLAB):
            ph = ppool_pm.tile([128, SW], f32, tag="pm")
            for j in range(DIM):
                for s in range(4):
                    nc.tensor.matmul(
                        ph[32*s:32*s+32, QS*j:QS*(j+1)],
                        wh[:, 32*j:32*j+32],
                        xth[:, 128*s+QS*sl:128*s+QS*(sl+1)],
                        start=True, stop=True,
                        tile_position=(0, 32*s),
                    )
            # s2 slab = diagp + pm*signpm: Act does the psum read + sign,
            # DVE adds the diagonal (pipelines across slabs)
            nc.scalar.activation(
                p1t[:, SW*sl:SW*(sl+1)].rearrange("p (q j) -> p q j", j=DIM),
                ph[:, :].rearrange("p (j q) -> p q j", j=DIM),
                Act.Copy, scale=signpm[:, :])
            nc.vector.tensor_tensor(s2[:, SW*sl:SW*(sl+1)],
                                    p1t[:, SW*sl:SW*(sl+1)],
                                    diagp[:, SW*sl:SW*(sl+1)], op=Alu.add)

        def build_wb_left(wb_t, s2_t, sl):
            """wb[:, 32q+0:16] = s2*signp (-> [Br;Bi]).  Alternate slabs go to
            Act (per-partition scale) and Pool (real-tensor multiply by the
            materialized sign plane -- Pool can't use scalar-broadcast ops)."""
            wbl = wb_t[:, :].rearrange("p (q j) -> p q j", j=32)
            src = s2_t[:, SW*sl:SW*(sl+1)].rearrange("p (q j) -> p q j", j=DIM)
            dst = wbl[:, sl*QS:(sl+1)*QS, 0:16]
            if sl % 2 == 0:
                nc.scalar.activation(dst, src, Act.Copy, scale=signp[:, :])
            else:
                nc.gpsimd.tensor_tensor(
                    dst, src,
                    signw[:, 0:SW].rearrange("p (q j) -> p q j", j=DIM),
                    op=Alu.mult)

        def build_wb_right(wb_t, s2_t, sl):
            """wb[:, 32q+16:32] = partition-half-swapped s2 (-> [-Bi;Br])."""
            wbw = wb_t[:, :].bitcast(u32).rearrange("p (q w) -> p q w", w=16)
            s2w = s2_t[:, :].bitcast(u32)
            nc.vector.stream_shuffle(
                wbw[:, sl*QS:(sl+1)*QS, 8:16],
                s2w[:, 8*sl*QS:8*(sl+1)*QS].rearrange("p (q w) -> p q w", w=8),
                mask=HSWAP)

        scl = [None, None]    # per slab-pair: [128, 64] tiles of +-1/s

        def norm_prefetch(s2_t, sl, sq_t, trp_t):
            """Normalizer for the NEXT step's cast: fro^2 of the state slab.
            Square on Pool (SBUF only); the (partition-block x j) double sum
            runs as 16 PSUM-accumulating matmuls over j-strided rhs slices
            with the sign-baked mask weights -- zero DVE cost; one batched
            recip per slab-pair then yields +-1/s."""
            c0, c1 = SW*sl, SW*(sl+1)
            # the normalizer only controls fp16 range, so sample every
            # FOURTH j-column: quarter the square/block-sum volume (stride 8
            # overflows fp16 on the even-E schedule; 4 holds with margin)
            JH2 = DIM // 4
            sc0 = JH2 * QS * sl
            seven = s2_t[:, c0:c1].rearrange(
                "p (q j four) -> p q j four", j=JH2, four=4)[:, :, :, 0]
            sqc = sq_t[:, sc0:sc0+JH2*QS].rearrange("p (q j) -> p q j", j=JH2)
            nc.vector.tensor_tensor(sqc, seven, seven, op=Alu.mult)
            slot = sl % 2
            if sl >= PREF_DVE_SLABS:
                sqj = sq_t[:, sc0:sc0+JH2*QS].rearrange(
                    "p (q j) -> p j q", j=JH2)
                for j in range(JH2):
                    nc.tensor.matmul(trp_t[:, QS*slot:QS*(slot+1)],
                                     maskbs[:, :], sqj[:, j, :],
                                     start=(j == 0), stop=(j == JH2 - 1))
            else:
                pr = wpool.tile([128, QS], f32, tag=f"pr{sl}")
                nc.vector.tensor_reduce(
                    pr[:, :],
                    sq_t[:, sc0:sc0+JH2*QS].rearrange("p (q j) -> p q j",
                                                      j=JH2),
                    axis=mybir.AxisListType.X, op=Alu.add)
                nc.tensor.matmul(trp_t[:, QS*slot:QS*(slot+1)], maskbsf[:, :],
                                 pr[:, :], start=True, stop=True)
            if slot == 1:
                # recip per slab-pair: ready mid-step, so next step's E-casts
                # for these slabs never wait on the tail slab's prefetch
                hp = sl // 2
                scl_t = wpool.tile([128, 2*QS], f32, tag=f"scl{hp}")
                nc.vector.reciprocal(scl_t[:, :], trp_t[:, :])
                scl[hp] = scl_t

        wb = spool.tile([128, 4096], f16, tag="wb")
        for sl in range(NSLAB):
            if sl % 2 == 0:
                build_wb_left(wb, s2, sl)
            else:
                wbl = wb[:, :].rearrange("p (q j) -> p q j", j=32)
                nc.vector.tensor_scalar_mul(
                    wbl[:, sl*QS:(sl+1)*QS, 0:16],
                    s2[:, SW*sl:SW*(sl+1)].rearrange("p (q j) -> p q j", j=DIM),
                    signp[:, :])
            build_wb_right(wb, s2, sl)

        # ---------------- phase 3: squaring iteration -----------------------
        # E-steps (normalized cast) on odd k; normalizer prefetched on even k.
        for k in range(ksteps):
            last = (k == ksteps - 1)
            # E-steps are STAGGERED by slab-pair (slabs 0-1 normalize on odd
            # steps, slabs 2-3 on even) so every step carries a uniform
            # DVE-cast/Act-cast/prefetch mix; the last step is unnormalized
            # for all slabs (the trace division absorbs the scale)
            s2n = spool.tile([128, 2048], f16, tag="s2")
            wbn = None if last else spool.tile([128, 4096], f16, tag="wb")
            sqn = trpn = None
            if k < ksteps - 2:
                sqn = wpool.tile([128, 1024], f16, tag="sq", name="sqn")
                trpn = ppool_sm.tile([128, 2*QS], f32, tag="sm", name="trpn")
            for sl in range(NSLAB):
                exact = (not last) and k >= 1 and \
                    (k % 2 == (1 if sl < 2 else 0))
                pref = (not last) and k + 1 < ksteps - 1 and \
                    ((k + 1) % 2 == (1 if sl < 2 else 0))
                q0 = sl * QS
                pm = ppool_pm.tile([128, SW], f32, tag="pm")
                for qq in range(QS):
                    q = q0 + qq
                    for s in range(4):
                        nc.tensor.matmul(
                            pm[32*s:32*s+32, 16*qq:16*qq+16],
                            wb[32*s:32*s+32, 32*q:32*q+32],
                            wb[32*s:32*s+32, 32*q:32*q+16],
                            start=True, stop=True,
                            tile_position=(32*s, 32*s))
                dst = s2n[:, SW*sl:SW*(sl+1)].rearrange("p (q j) -> p q j", j=DIM)
                src = pm[:, :].rearrange("p (q j) -> p q j", j=DIM)
                if exact:
                    # cast: s2' = pm * (+-1/s), per-quad scale -> DVE TT
                    nc.vector.tensor_tensor(
                        dst, src,
                        scl[sl // 2][:, QS*(sl % 2):QS*(sl % 2 + 1)]
                            .unsqueeze(-1).broadcast_to([128, QS, DIM]),
                        op=Alu.mult)
                elif last and sl % 2 == 1:
                    # last step: DVE is idle, split the casts with Act
                    nc.vector.tensor_scalar_mul(dst, src, signp[:, :])
                else:
                    # cast: s2' = pm * sign (no normalization this step)
                    nc.scalar.activation(dst, src, Act.Copy, scale=signp[:, :])
                if not last:
                    with tc.high_priority(offset=WB_PRIO):
                        build_wb_right(wbn, s2n, sl)
                        build_wb_left(wbn, s2n, sl)
                    if pref:
                        norm_prefetch(s2n, sl, sqn, trpn)
            s2 = s2n
            if not last:
                wb = wbn

        # ---------------- phase 4: finish (per-slab for pipelining) ---------
        # pos/ea2/trace fold their j-rowsums into PSUM-accumulating matmuls
        # over j-strided state slices: no DVE reduces at all in the tail
        trf = ppool_sm.tile([128, 128], f32, tag="sm")
        pos = ppool_sm.tile([128, 128], f32, tag="sm")
        ea2 = ppool_sm.tile([128, 128], f32, tag="sm")
        def s2j(sl):
            return s2[:, SW*sl:SW*(sl+1)].rearrange("p (q j) -> p j q", j=DIM)
        for w_t, out_t in ((wtrc, trf), (wpos, pos), (wea2, ea2)):
            for sl in range(NSLAB):
                qs = slice(QS*sl, QS*(sl+1))
                for j in range(DIM):
                    wsl = w_t[:, 128*j:128*(j+1)] if w_t is wtrc else w_t[:, :]
                    nc.tensor.matmul(out_t[:, qs], wsl, s2j(sl)[:, j, :],
                                     start=(j == 0), stop=(j == DIM - 1))
        # the rest is column-sliceable; per-slab chains run concurrently on
        # DVE and Pool (PSUM readers stay on DVE; Pool's chain avoids scalar
        # ops -- TensorScalarPtr is illegal there -- via LAMW); the output
        # DMA is split so the first half's latency hides under the second
        invt = wpool.tile([128, 128], f32, tag="invt")
        posn = wpool.tile([128, 128], f32, tag="posn")
        ea2s = wpool.tile([128, 128], f32, tag="ea2s")
        terr = wpool.tile([128, 128], f32, tag="terr")
        t2 = wpool.tile([128, 128], f32, tag="t2")
        np2 = wpool.tile([128, 128], f32, tag="np2")
        r1 = wpool.tile([128, 128], f32, tag="r1")
        r = wpool.tile([128, 128], f32, tag="r")
        outv = wpool.tile([128, 2], f32, tag="outv")
        lamw = wpool.tile([128, 64], f32, tag="lamw")
        nc.gpsimd.memset(lamw[:, :], -LAM)
        for hf in range(2):
            cs = slice(64*hf, 64*hf+64)
            te = nc.vector if hf == 0 else nc.gpsimd
            nc.vector.reciprocal(invt[:, cs], trf[:, cs])
            nc.vector.tensor_tensor(posn[:, cs], pos[:, cs], invt[:, cs],
                                    op=Alu.mult)
            nc.vector.tensor_tensor(ea2s[:, cs], ea2[:, cs], invt[:, cs],
                                    op=Alu.mult)
            # r = terr^2 + ea2s - LAM*posn^2   (LAM baked into WEA2)
            te.tensor_tensor(terr[:, cs], posn[:, cs], xblk[:, cs],
                             op=Alu.subtract)
            te.tensor_tensor(t2[:, cs], terr[:, cs], terr[:, cs], op=Alu.mult)
            if hf == 0:
                nc.vector.scalar_tensor_tensor(np2[:, cs], posn[:, cs], -LAM,
                                               posn[:, cs],
                                               op0=Alu.mult, op1=Alu.mult)
            else:
                nc.gpsimd.tensor_tensor(np2[:, cs], posn[:, cs], posn[:, cs],
                                        op=Alu.mult)
                nc.gpsimd.tensor_tensor(np2[:, cs], np2[:, cs],
                                        lamw[:, :], op=Alu.mult)
            te.tensor_tensor(r1[:, cs], t2[:, cs], ea2s[:, cs], op=Alu.add)
            te.tensor_tensor(r[:, cs], r1[:, cs], np2[:, cs], op=Alu.add)
            nc.vector.tensor_reduce(outv[:, hf:hf+1], r[:, cs],
                                    axis=mybir.AxisListType.X, op=Alu.add)
            nc.sync.dma_start(d_out[:, hf:hf+1], outv[:, hf:hf+1])
    nc.compile()
    return nc


def kernel(A_real, A_imag, X):
    from concourse.bass_utils import run_bass_kernel_spmd

    per_core = _build_host_tensors(
        np.asarray(A_real, np.float32), np.asarray(A_imag, np.float32),
        np.asarray(X, np.float32))

    if "nc" not in _prog_cache:
        _prog_cache["nc"] = build_program()
    nc = _prog_cache["nc"]

    in_maps = [per_core[c] for c in range(NCORES)]
    res = run_bass_kernel_spmd(nc, in_maps, list(range(NCORES)))
    total = 0.0
    for c in range(NCORES):
        total += float(np.asarray(res.results[c]["out"], np.float64).sum())
    loss = total / N
    return np.float32(loss)
